# revision 8
# baseline (speedup 1.0000x reference)
"""DD-RoPE kernel for 8x TRN2 NeuronCores — pipelined "t-on-partitions" v3.

Reference computation (B=4, T=4096, D=2048, P=256):
    deltas = einsum('btd,pd->btp', x, W) + b     # (B, T, P)
    angles = cumsum(deltas, axis=1)
    out = concat([x1*cos(a) - x2*sin(a), x2*cos(a) + x1*sin(a), x[..., 512:]], -1)

Sharding: 8 shards = 4 batches x 2 T-halves (2048 steps each), data-parallel.
The cumsum is split into independent 128-step blocks via host-computed fp64
block bases (exact cumulative angle at each 128-step boundary) injected on
device through a rank-4 affine matmul, so no cross-core communication and
bounded within-block drift.

v3 design notes (vs the 71us v1/v2):
  - measurement showed steady state was DMA-bound: 1.5 MiB/pair (xt 1 MiB +
    x12 0.25 + out 0.25) at ~360 GB/s = 4.4us/pair vs 3.9us of PE work.
    v3 DELETES the x12 stream entirely: the range-reduced angles are
    transposed on the PE (2 cheap [128,128] transposes per block) and the
    trig + rotation run in [p, t] layout, where x1^T/x2^T are exactly the
    dc=0..3 chunks of the xt tiles already resident for the delta matmuls.
    Per-pair DMA drops to 1.25 MiB = 3.6us < PE 4.2us -> PE-bound.
  - the TRN2 PE p-state ramp (full 2.4 GHz only after 3us of continuous
    execution, reset on idle) is handled by (a) warming the PE with dummy
    matmuls during the DMA prologue and (b) never letting the PE starve:
    the first real matmul is gated on a whole 512 KiB xt block (not a
    fine-grained early chunk that would run dry and reset the ramp).
  - Sin activation table load (1.3us) pulled into the prologue by a dummy
    Sin; x12/out/const DMAs issue from the idle GpSimd sequencer (25ns per
    DMA vs 565ns on SP) so SP only sequences the xt/w stream.
  - tapered tail: the last two blocks run as single-block (256-wide) chains.

Engine budget per pair (cost model): PE 32 MM + 4 angle MM + 4 transposes
= 4.2us; Scalar d16/a_s/sin/cos = 2.8us; DVE rs/|rs|/6 rot = 3.3us;
DMA 1.25 MiB = 3.6us.
"""

import sys

if "/opt/trn_rl_repo" not in sys.path:
    sys.path.insert(0, "/opt/trn_rl_repo")

from contextlib import ExitStack

import numpy as np

import concourse.bacc as bacc
import concourse.bass as bass
import concourse.mybir as mybir
import concourse.tile as tile
from concourse.bass_utils import run_bass_kernel_spmd

F32 = mybir.dt.float32
F16 = mybir.dt.float16
ADD = mybir.AluOpType.add
SUB = mybir.AluOpType.subtract
MULT = mybir.AluOpType.mult
MAX = mybir.AluOpType.max
IDENT = mybir.ActivationFunctionType.Identity
SIN = mybir.ActivationFunctionType.Sin

D = 2048          # input feature dim (contraction)
P = 256           # delta-pairs dim
ROT = 2 * P       # rotated columns (512)
TL = 2048         # time steps per shard
BK = 128          # cumsum block (base injection granularity)
NBK = TL // BK    # blocks per shard (16)
KC = D // 128     # contraction chunks (16)
NPAIR = NBK // 2  # row-pairs in the xt dram layout (8)
N_CORES = 8

# pipeline items: (block offset, blocks in item) — tapered single-block tail
ITEMS = [(0, 2), (2, 2), (4, 2), (6, 2), (8, 2), (10, 2), (12, 2),
         (14, 1), (15, 1)]
# outT column offset per item: 4*nb*128 columns each
ITEM_OFF = []
_off = 0
for _bo, _nb in ITEMS:
    ITEM_OFF.append(_off)
    _off += 4 * _nb * 128
OUT_COLS = _off   # 8192

N_WARM_MM = 12    # dummy matmuls to ramp the PE p-state during the prologue

MAGIC = 12582912.0          # 1.5 * 2**23: fp32 round-to-int magic constant
SCALE_2PI = 6.28310         # slightly < 2*pi so Sin args stay inside [-pi, pi]
HALF_PI = 1.5707964


def build_program() -> bass.Bass:
    nc = bacc.Bacc("TRN2", target_bir_lowering=False, debug=False)

    # x^T tiles: [r*128 + dp, (bkl*KC + dc)*128 + tl] = xs[(2r+bkl)*128+tl,
    #                                                      dc*128 + dp]
    xt = nc.dram_tensor("xt", [NPAIR * 128, 2 * KC * 128], F16,
                        kind="ExternalInput").ap()
    # W, d-chunks along free: [128 d-part, dc*P + p] fp16
    w = nc.dram_tensor("w", [128, KC * P], F16, kind="ExternalInput").ap()
    # upper-triangular ones (u[t, t'] = 1 iff t <= t')
    u = nc.dram_tensor("u", [128, 128], F16, kind="ExternalInput").ap()
    # identity (for PE transposes)
    ident = nc.dram_tensor("ident", [128, 128], F16,
                           kind="ExternalInput").ap()
    # affine stationary: rows [ones, ones, ramp(1..128), ramp]
    afs = nc.dram_tensor("afs", [4, 128], F16, kind="ExternalInput").ap()
    # affine moving: rows [base_hi[bk,p], base_lo, b_hi, b_lo], bk-major
    afm = nc.dram_tensor("afm", [4, NBK * P], F16, kind="ExternalInput").ap()
    # rotated output in [p, t] layout: [q, item_off + (h*2 + c)*nb*128
    #                                       + b*128 + t]
    # (q = p%128, c = p//128, h = rotation half, b = block-in-item, t local)
    outT = nc.dram_tensor("outT", [128, OUT_COLS], F16,
                          kind="ExternalOutput").ap()

    with tile.TileContext(nc) as tc, ExitStack() as ctx:
        const_pool = ctx.enter_context(tc.tile_pool(name="const", bufs=1))
        w_pool = ctx.enter_context(tc.tile_pool(name="w", bufs=1))
        xt_pool = ctx.enter_context(tc.tile_pool(name="xt", bufs=3))
        dp_pool = ctx.enter_context(
            tc.tile_pool(name="dp_psum", bufs=3, space="PSUM"))
        ang_pool = ctx.enter_context(
            tc.tile_pool(name="ang_psum", bufs=2, space="PSUM"))
        rst_pool = ctx.enter_context(
            tc.tile_pool(name="rst_psum", bufs=2, space="PSUM"))
        junk_pool = ctx.enter_context(
            tc.tile_pool(name="junk_psum", bufs=1, space="PSUM"))
        d16_pool = ctx.enter_context(tc.tile_pool(name="d16", bufs=2))
        a32_pool = ctx.enter_context(tc.tile_pool(name="a32", bufs=2))
        trig_pool = ctx.enter_context(tc.tile_pool(name="trig", bufs=2))
        rot_pool = ctx.enter_context(tc.tile_pool(name="rot", bufs=2))
        out_pool = ctx.enter_context(tc.tile_pool(name="out", bufs=2))

        # --- prologue ----------------------------------------------------
        w_sb = w_pool.tile([128, KC * P], F16, tag="w")
        # first quarter of w (dc 0..3) gates the very first matmul
        nc.sync.dma_start(w_sb[:, 0:4 * P], w[:, 0:4 * P])

        u_sb = const_pool.tile([128, 128], F16, tag="u")
        id_sb = const_pool.tile([128, 128], F16, tag="ident")
        afs_sb = const_pool.tile([4, 128], F16, tag="afs")
        afm_sb = const_pool.tile([4, NBK * P], F16, tag="afm")
        nc.gpsimd.dma_start(u_sb[:], u[:])
        nc.gpsimd.dma_start(id_sb[:], ident[:])
        nc.gpsimd.dma_start(afs_sb[:], afs[:])
        nc.gpsimd.dma_start(afm_sb[:], afm[:])
        magic_sb = const_pool.tile([128, 1], F32, tag="magic")
        nc.gpsimd.memset(magic_sb[:], MAGIC)
        hpi_sb = const_pool.tile([128, 1], F32, tag="hpi")
        nc.gpsimd.memset(hpi_sb[:], HALF_PI)
        # dummy Sin pulls the 1.3us ACT_TABLE_LOAD into the prologue
        warm_sb = const_pool.tile([128, 1], F16, tag="warm")
        nc.gpsimd.memset(warm_sb[:], 0.0)
        warm2_sb = const_pool.tile([128, 1], F16, tag="warm2")
        nc.scalar.activation(warm2_sb[:], warm_sb[:], SIN)
        # dummy matmuls ramp the PE p-state while the first xt block lands
        junk_sb = const_pool.tile([128, 512], F16, tag="junk")
        nc.gpsimd.memset(junk_sb[:], 0.0)
        junk_ps = junk_pool.tile([128, 512], F32, tag="junkp")
        for _ in range(N_WARM_MM):
            nc.tensor.matmul(junk_ps[:], junk_sb[:, 0:128], junk_sb[:],
                             start=True, stop=True)

        def issue_in_dmas(it):
            bo, nb = ITEMS[it]
            r, lo = bo // 2, bo % 2
            rows = slice(r * 128, (r + 1) * 128)
            xtg = xt_pool.tile([128, nb * KC * 128], F16, tag="xt")
            if it == 0:
                # one DMA per block: the first matmul waits for a whole
                # block (PE never starves mid-block), the second block
                # streams while block 0 computes
                nc.sync.dma_start(xtg[:, 0:KC * 128], xt[rows, 0:KC * 128])
                nc.sync.dma_start(xtg[:, KC * 128:2 * KC * 128],
                                  xt[rows, KC * 128:2 * KC * 128])
            else:
                xsl = slice(lo * KC * 128, (lo + nb) * KC * 128)
                nc.sync.dma_start(xtg[:], xt[rows, xsl])
            return xtg

        def stage_deltas(it, xtg):
            bo, nb = ITEMS[it]
            wid = nb * P
            dp = dp_pool.tile([128, wid], F32, tag="dp")
            for bkl in range(nb):
                sl = slice(bkl * P, (bkl + 1) * P)
                for dc in range(KC):
                    nc.tensor.matmul(
                        dp[:, sl],
                        xtg[:, (bkl * KC + dc) * 128:(bkl * KC + dc + 1) * 128],
                        w_sb[:, dc * P:(dc + 1) * P],
                        start=(dc == 0), stop=(dc == KC - 1))
            d16 = d16_pool.tile([128, wid], F16, tag="d16")
            nc.scalar.activation(d16[:], dp[:], IDENT)
            return d16

        def stage_back(it, d16, xtg):
            """Angle matmuls + trig + [p,t] rotation + out DMA for item."""
            bo, nb = ITEMS[it]
            wid = nb * P
            ang = ang_pool.tile([128, wid], F32, tag="ang")
            for bkl in range(nb):
                bk = bo + bkl
                sl = slice(bkl * P, (bkl + 1) * P)
                nc.tensor.matmul(ang[:, sl], u_sb[:], d16[:, sl],
                                 start=True, stop=False)
                nc.tensor.matmul(ang[:, sl], afs_sb[:],
                                 afm_sb[:, bk * P:(bk + 1) * P],
                                 start=False, stop=True)

            # range reduction (turns): rs = y - round(y) in [-0.5, 0.5]
            a_s = a32_pool.tile([128, wid], F32, tag="a_s")
            nc.scalar.activation(a_s[:], ang[:], IDENT,
                                 bias=magic_sb[:], scale=-1.0)
            rs = trig_pool.tile([128, wid], F16, tag="rs")
            nc.vector.scalar_tensor_tensor(rs[:], a_s[:], MAGIC, ang[:],
                                           op0=SUB, op1=ADD)

            # transpose rs to [p, t]: per (block, p-chunk) 128x128 PE
            # transpose; rsT cols = (b, c, t)
            rst = rst_pool.tile([128, nb * 2 * 128], F16, tag="rst")
            for bkl in range(nb):
                for pc in range(2):
                    nc.tensor.transpose(
                        rst[:, (bkl * 2 + pc) * 128:(bkl * 2 + pc + 1) * 128],
                        rs[:, bkl * P + pc * 128:bkl * P + (pc + 1) * 128],
                        id_sb[:])

            sn = trig_pool.tile([128, nb * 2 * 128], F16, tag="sn")
            nc.scalar.activation(sn[:], rst[:], SIN, scale=SCALE_2PI)
            # cos(2pi*y) = sin(pi/2 - 2pi*|rs|), same reduction
            ra = trig_pool.tile([128, nb * 2 * 128], F16, tag="ra")
            nc.scalar.activation(ra[:], rst[:],
                                 mybir.ActivationFunctionType.Abs)
            cs = trig_pool.tile([128, nb * 2 * 128], F16, tag="cs")
            nc.scalar.activation(cs[:], ra[:], SIN,
                                 scale=-SCALE_2PI, bias=hpi_sb[:])

            # rotation in [p, t]: x1^T/x2^T are xt chunks dc 0..1 / 2..3
            xv = xtg[:].rearrange("q (b k t) -> q b k t",
                                  b=nb, k=KC, t=128)
            x1 = xv[:, :, 0:2, :]
            x2 = xv[:, :, 2:4, :]
            snv = sn[:].rearrange("q (b c t) -> q b c t", b=nb, c=2, t=128)
            csv = cs[:].rearrange("q (b c t) -> q b c t", b=nb, c=2, t=128)
            wid2 = nb * 2 * 128
            o = out_pool.tile([128, 2 * wid2], F16, tag="o")
            o1 = o[:, 0:wid2].rearrange("q (c b t) -> q b c t",
                                        c=2, b=nb, t=128)
            o2 = o[:, wid2:2 * wid2].rearrange("q (c b t) -> q b c t",
                                               c=2, b=nb, t=128)
            t1 = rot_pool.tile([128, wid2], F16, tag="t1")
            t1v = t1[:].rearrange("q (b c t) -> q b c t", b=nb, c=2, t=128)
            nc.vector.tensor_mul(t1v, x1, csv)
            t2 = rot_pool.tile([128, wid2], F16, tag="t2")
            t2v = t2[:].rearrange("q (b c t) -> q b c t", b=nb, c=2, t=128)
            nc.vector.tensor_mul(t2v, x2, snv)
            nc.vector.tensor_sub(o1, t1v, t2v)
            t3 = rot_pool.tile([128, wid2], F16, tag="t3")
            t3v = t3[:].rearrange("q (b c t) -> q b c t", b=nb, c=2, t=128)
            nc.vector.tensor_mul(t3v, x2, csv)
            t4 = rot_pool.tile([128, wid2], F16, tag="t4")
            t4v = t4[:].rearrange("q (b c t) -> q b c t", b=nb, c=2, t=128)
            nc.vector.tensor_mul(t4v, x1, snv)
            nc.vector.tensor_add(o2, t3v, t4v)

            off = ITEM_OFF[it]
            nc.gpsimd.dma_start(outT[:, off:off + 4 * nb * 128], o[:])

        # remaining w quarters right after the first xt DMAs in SP order
        def issue_w_rest():
            for q in range(1, 4):
                nc.sync.dma_start(w_sb[:, q * 4 * P:(q + 1) * 4 * P],
                                  w[:, q * 4 * P:(q + 1) * 4 * P])

        pend = None  # (it, d16, xtg) awaiting its back stage
        for it in range(len(ITEMS)):
            xtg = issue_in_dmas(it)
            if it == 0:
                issue_w_rest()
            d16 = stage_deltas(it, xtg)
            if pend is not None:
                stage_back(*pend)
            pend = (it, d16, xtg)
        stage_back(*pend)

    nc.compile()
    return nc


_NC_CACHE: dict = {}


def _get_nc():
    if "nc" not in _NC_CACHE:
        _NC_CACHE["nc"] = build_program()
    return _NC_CACHE["nc"]


def prepare_weights(W: np.ndarray, b: np.ndarray):
    inv2pi = 1.0 / (2.0 * np.pi)
    Wt = W.astype(np.float64).T * inv2pi                       # [D, P]
    wh = Wt.astype(np.float16)
    bt = b.astype(np.float64) * inv2pi                         # [P]
    bh = bt.astype(np.float16)
    bl = (bt - bh.astype(np.float64)).astype(np.float16)
    # [D, P] -> [128, KC*P] with d-chunks along the free dim
    w_in = np.ascontiguousarray(
        wh.reshape(KC, 128, P).transpose(1, 0, 2).reshape(128, KC * P))
    # Bases must come from the FULL-precision weights so each 128-step block
    # restarts at the reference-exact angle: the device's fp16-W error then
    # only drifts within one block instead of accumulating across the shard.
    return w_in, bh, bl, Wt, bt


def make_in_maps(x: np.ndarray, W: np.ndarray, b: np.ndarray):
    B, T, _ = x.shape
    w_in, bh, bl, w_eff, b_eff = prepare_weights(W, b)

    u_in = np.triu(np.ones((128, 128), np.float16))
    id_in = np.eye(128, dtype=np.float16)
    afs_in = np.stack([
        np.ones(128, np.float16), np.ones(128, np.float16),
        np.arange(1, 129, dtype=np.float16),
        np.arange(1, 129, dtype=np.float16)])

    # fp64 cumulative angle at every 128-step boundary, per batch (turns)
    nblk = T // BK                                              # 32
    xblk = x.reshape(B, nblk, BK, D).sum(axis=2, dtype=np.float64)
    dblk = xblk @ w_eff + BK * b_eff                            # [B, 32, P]
    bases = np.zeros((B, nblk, P))
    np.cumsum(dblk[:, :-1], axis=1, out=bases[:, 1:])           # exclusive

    in_maps = []
    for c in range(N_CORES):
        bb, hh = c // 2, c % 2
        xs = x[bb, hh * TL:(hh + 1) * TL, :].astype(np.float16)  # [TL, D]
        # xt: [r*128 + dp, (bkl*KC + dc)*128 + tl]
        xt_in = np.ascontiguousarray(
            xs.reshape(NPAIR, 2, BK, KC, 128).transpose(0, 4, 1, 3, 2)
            .reshape(NPAIR * 128, 2 * KC * 128))
        bs = bases[bb, hh * NBK:(hh + 1) * NBK]                 # [NBK, P]
        bs_hi = bs.astype(np.float16)
        bs_lo = (bs - bs_hi.astype(np.float64)).astype(np.float16)
        afm_in = np.stack([
            bs_hi.reshape(NBK * P), bs_lo.reshape(NBK * P),
            np.tile(bh, NBK), np.tile(bl, NBK)])
        in_maps.append({
            "xt": xt_in, "w": w_in, "u": u_in, "ident": id_in,
            "afs": afs_in, "afm": np.ascontiguousarray(afm_in),
        })
    return in_maps


def assemble_output(x: np.ndarray, results) -> np.ndarray:
    B, T, Din = x.shape
    out = np.empty((B, T, Din), np.float32)
    out[:, :, ROT:] = x[:, :, ROT:]
    for c in range(N_CORES):
        bb, hh = c // 2, c % 2
        r = results[c]["outT"]                                # [128, 8192]
        blk = np.empty((TL, ROT), np.float32)
        for it, (bo, nb) in enumerate(ITEMS):
            off = ITEM_OFF[it]
            seg = r[:, off:off + 4 * nb * 128]                # [q, h*c*b*t]
            seg = seg.reshape(128, 2, 2, nb, 128)             # q h c b t
            # rotated[(bo+b)*128 + t, h*256 + c*128 + q]
            seg = seg.transpose(3, 4, 1, 2, 0).reshape(nb * 128, ROT)
            blk[bo * 128:(bo + nb) * 128] = seg
        out[bb, hh * TL:(hh + 1) * TL, :ROT] = blk
    return out


def kernel(x: np.ndarray, W: np.ndarray, b: np.ndarray) -> np.ndarray:
    nc = _get_nc()
    in_maps = make_in_maps(x, W, b)
    res = run_bass_kernel_spmd(nc, in_maps, list(range(N_CORES)))
    return assemble_output(x, res.results)


# revision 14
# speedup vs baseline: 1.0036x; 1.0036x over previous
"""DD-RoPE kernel for 8x TRN2 NeuronCores — pipelined "t-on-partitions" v3.

Reference computation (B=4, T=4096, D=2048, P=256):
    deltas = einsum('btd,pd->btp', x, W) + b     # (B, T, P)
    angles = cumsum(deltas, axis=1)
    out = concat([x1*cos(a) - x2*sin(a), x2*cos(a) + x1*sin(a), x[..., 512:]], -1)

Sharding: 8 shards = 4 batches x 2 T-halves (2048 steps each), data-parallel.
The cumsum is split into independent 128-step blocks via host-computed fp64
block bases (exact cumulative angle at each 128-step boundary) injected on
device through a rank-4 affine matmul, so no cross-core communication and
bounded within-block drift.

v3 design notes (vs the 71us v1/v2):
  - measurement showed steady state was DMA-bound: 1.5 MiB/pair (xt 1 MiB +
    x12 0.25 + out 0.25) at ~360 GB/s = 4.4us/pair vs 3.9us of PE work.
    v3 DELETES the x12 stream entirely: the range-reduced angles are
    transposed on the PE (2 cheap [128,128] transposes per block) and the
    trig + rotation run in [p, t] layout, where x1^T/x2^T are exactly the
    dc=0..3 chunks of the xt tiles already resident for the delta matmuls.
    Per-pair DMA drops to 1.25 MiB = 3.6us < PE 4.2us -> PE-bound.
  - the TRN2 PE p-state ramp (full 2.4 GHz only after 3us of continuous
    execution, reset on idle) is handled by (a) warming the PE with dummy
    matmuls during the DMA prologue and (b) never letting the PE starve:
    the first real matmul is gated on a whole 512 KiB xt block (not a
    fine-grained early chunk that would run dry and reset the ramp).
  - Sin activation table load (1.3us) pulled into the prologue by a dummy
    Sin; x12/out/const DMAs issue from the idle GpSimd sequencer (25ns per
    DMA vs 565ns on SP) so SP only sequences the xt/w stream.
  - tapered tail: the last two blocks run as single-block (256-wide) chains.

Engine budget per pair (cost model): PE 32 MM + 4 angle MM + 4 transposes
= 4.2us; Scalar d16/a_s/sin/cos = 2.8us; DVE rs/|rs|/6 rot = 3.3us;
DMA 1.25 MiB = 3.6us.
"""

import sys

if "/opt/trn_rl_repo" not in sys.path:
    sys.path.insert(0, "/opt/trn_rl_repo")

from contextlib import ExitStack

import numpy as np

import concourse.bacc as bacc
import concourse.bass as bass
import concourse.mybir as mybir
import concourse.tile as tile
from concourse.bass_utils import run_bass_kernel_spmd

F32 = mybir.dt.float32
F16 = mybir.dt.float16
ADD = mybir.AluOpType.add
SUB = mybir.AluOpType.subtract
MULT = mybir.AluOpType.mult
MAX = mybir.AluOpType.max
IDENT = mybir.ActivationFunctionType.Identity
SIN = mybir.ActivationFunctionType.Sin

D = 2048          # input feature dim (contraction)
P = 256           # delta-pairs dim
ROT = 2 * P       # rotated columns (512)
TL = 2048         # time steps per shard
BK = 128          # cumsum block (base injection granularity)
NBK = TL // BK    # blocks per shard (16)
KC = D // 128     # contraction chunks (16)
NPAIR = NBK // 2  # row-pairs in the xt dram layout (8)
N_CORES = 8

# pipeline items: (block offset, blocks in item) — tapered single-block tail
ITEMS = [(0, 2), (2, 2), (4, 2), (6, 2), (8, 2), (10, 2), (12, 2),
         (14, 1), (15, 1)]
# outT column offset per item: 4*nb*128 columns each
ITEM_OFF = []
_off = 0
for _bo, _nb in ITEMS:
    ITEM_OFF.append(_off)
    _off += 4 * _nb * 128
OUT_COLS = _off   # 8192

N_WARM_MM = 10    # dummy matmuls to ramp the PE p-state during the prologue

MAGIC = 12582912.0          # 1.5 * 2**23: fp32 round-to-int magic constant
SCALE_2PI = 6.28310         # slightly < 2*pi so Sin args stay inside [-pi, pi]
HALF_PI = 1.5707964


def build_program() -> bass.Bass:
    nc = bacc.Bacc("TRN2", target_bir_lowering=False, debug=False)

    # x^T tiles: [r*128 + dp, (bkl*KC + dc)*128 + tl] = xs[(2r+bkl)*128+tl,
    #                                                      dc*128 + dp]
    xt = nc.dram_tensor("xt", [NPAIR * 128, 2 * KC * 128], F16,
                        kind="ExternalInput").ap()
    # W, d-chunks along free: [128 d-part, dc*P + p] fp16
    w = nc.dram_tensor("w", [128, KC * P], F16, kind="ExternalInput").ap()
    # upper-triangular ones (u[t, t'] = 1 iff t <= t')
    u = nc.dram_tensor("u", [128, 128], F16, kind="ExternalInput").ap()
    # identity (for PE transposes)
    ident = nc.dram_tensor("ident", [128, 128], F16,
                           kind="ExternalInput").ap()
    # affine stationary: rows [ones, ones, ramp(1..128), ramp]
    afs = nc.dram_tensor("afs", [4, 128], F16, kind="ExternalInput").ap()
    # affine moving: rows [base_hi[bk,p], base_lo, b_hi, b_lo], bk-major
    afm = nc.dram_tensor("afm", [4, NBK * P], F16, kind="ExternalInput").ap()
    # rotated output in [p, t] layout: [q, item_off + (h*2 + c)*nb*128
    #                                       + b*128 + t]
    # (q = p%128, c = p//128, h = rotation half, b = block-in-item, t local)
    outT = nc.dram_tensor("outT", [128, OUT_COLS], F16,
                          kind="ExternalOutput").ap()

    with tile.TileContext(nc) as tc, ExitStack() as ctx:
        const_pool = ctx.enter_context(tc.tile_pool(name="const", bufs=1))
        w_pool = ctx.enter_context(tc.tile_pool(name="w", bufs=1))
        xt_pool = ctx.enter_context(tc.tile_pool(name="xt", bufs=6))
        dp_pool = ctx.enter_context(
            tc.tile_pool(name="dp_psum", bufs=3, space="PSUM"))
        ang_pool = ctx.enter_context(
            tc.tile_pool(name="ang_psum", bufs=2, space="PSUM"))
        rst_pool = ctx.enter_context(
            tc.tile_pool(name="rst_psum", bufs=2, space="PSUM"))
        junk_pool = ctx.enter_context(
            tc.tile_pool(name="junk_psum", bufs=1, space="PSUM"))
        d16_pool = ctx.enter_context(tc.tile_pool(name="d16", bufs=2))
        a32_pool = ctx.enter_context(tc.tile_pool(name="a32", bufs=2))
        trig_pool = ctx.enter_context(tc.tile_pool(name="trig", bufs=2))
        rot_pool = ctx.enter_context(tc.tile_pool(name="rot", bufs=2))
        out_pool = ctx.enter_context(tc.tile_pool(name="out", bufs=2))

        # --- prologue ----------------------------------------------------
        # junk memset + dummy matmuls FIRST, fed from the otherwise-idle
        # DVE queue so the PE p-state ramp starts right after the barrier
        # (a gpsimd-fed memset would chain the PE behind the whole gpsimd
        # prologue through Tile's coalesced per-engine semaphores)
        junk_sb = const_pool.tile([128, 512], F16, tag="junk")
        nc.vector.memset(junk_sb[:], 0.0)
        junk_ps = junk_pool.tile([128, 512], F32, tag="junkp")
        for _ in range(N_WARM_MM):
            nc.tensor.matmul(junk_ps[:], junk_sb[:, 0:128], junk_sb[:],
                             start=True, stop=True)

        # critical first transfers from the Activation queue (parallel to
        # SP's preamble): first w quarter + xt block 0 (in issue_in_dmas)
        w_sb = w_pool.tile([128, KC * P], F16, tag="w")
        nc.scalar.dma_start(w_sb[:, 0:4 * P], w[:, 0:4 * P])

        u_sb = const_pool.tile([128, 128], F16, tag="u")
        id_sb = const_pool.tile([128, 128], F16, tag="ident")
        afs_sb = const_pool.tile([4, 128], F16, tag="afs")
        afm_sb = const_pool.tile([4, NBK * P], F16, tag="afm")
        magic_sb = const_pool.tile([128, 1], F32, tag="magic")
        nc.gpsimd.memset(magic_sb[:], MAGIC)
        hpi_sb = const_pool.tile([128, 1], F32, tag="hpi")
        nc.gpsimd.memset(hpi_sb[:], HALF_PI)
        # dummy Sin pulls the 1.3us ACT_TABLE_LOAD into the prologue
        warm_sb = const_pool.tile([128, 1], F16, tag="warm")
        nc.gpsimd.memset(warm_sb[:], 0.0)
        warm2_sb = const_pool.tile([128, 1], F16, tag="warm2")
        nc.scalar.activation(warm2_sb[:], warm_sb[:], SIN)

        def issue_in_dmas(it):
            bo, nb = ITEMS[it]
            r, lo = bo // 2, bo % 2
            rows = slice(r * 128, (r + 1) * 128)
            xtg = xt_pool.tile([128, nb * KC * 128], F16, tag="xt")
            if it == 0:
                # one DMA per block from the Activation queue: the first
                # matmul waits for a whole block so the PE never starves
                # mid-block
                nc.scalar.dma_start(xtg[:, 0:KC * 128], xt[rows, 0:KC * 128])
                nc.scalar.dma_start(xtg[:, KC * 128:2 * KC * 128],
                                    xt[rows, KC * 128:2 * KC * 128])
            else:
                xsl = slice(lo * KC * 128, (lo + nb) * KC * 128)
                nc.sync.dma_start(xtg[:], xt[rows, xsl])
            return xtg

        def stage_deltas(it, xtg):
            bo, nb = ITEMS[it]
            wid = nb * P
            dp = dp_pool.tile([128, wid], F32, tag="dp")
            for bkl in range(nb):
                sl = slice(bkl * P, (bkl + 1) * P)
                for dc in range(KC):
                    nc.tensor.matmul(
                        dp[:, sl],
                        xtg[:, (bkl * KC + dc) * 128:(bkl * KC + dc + 1) * 128],
                        w_sb[:, dc * P:(dc + 1) * P],
                        start=(dc == 0), stop=(dc == KC - 1))
            d16 = d16_pool.tile([128, wid], F16, tag="d16")
            nc.scalar.activation(d16[:], dp[:], IDENT)
            return d16

        def stage_back(it, d16, xtg):
            """Angle matmuls + trig + [p,t] rotation + out DMA for item."""
            bo, nb = ITEMS[it]
            wid = nb * P
            ang = ang_pool.tile([128, wid], F32, tag="ang")
            for bkl in range(nb):
                bk = bo + bkl
                sl = slice(bkl * P, (bkl + 1) * P)
                nc.tensor.matmul(ang[:, sl], u_sb[:], d16[:, sl],
                                 start=True, stop=False)
                nc.tensor.matmul(ang[:, sl], afs_sb[:],
                                 afm_sb[:, bk * P:(bk + 1) * P],
                                 start=False, stop=True)

            # range reduction (turns): rs = y - round(y) in [-0.5, 0.5]
            a_s = a32_pool.tile([128, wid], F32, tag="a_s")
            nc.scalar.activation(a_s[:], ang[:], IDENT,
                                 bias=magic_sb[:], scale=-1.0)
            rs = trig_pool.tile([128, wid], F16, tag="rs")
            nc.vector.scalar_tensor_tensor(rs[:], a_s[:], MAGIC, ang[:],
                                           op0=SUB, op1=ADD)

            # transpose rs to [p, t]: per (block, p-chunk) 128x128 PE
            # transpose; rsT cols = (b, c, t)
            rst = rst_pool.tile([128, nb * 2 * 128], F16, tag="rst")
            for bkl in range(nb):
                for pc in range(2):
                    nc.tensor.transpose(
                        rst[:, (bkl * 2 + pc) * 128:(bkl * 2 + pc + 1) * 128],
                        rs[:, bkl * P + pc * 128:bkl * P + (pc + 1) * 128],
                        id_sb[:])

            sn = trig_pool.tile([128, nb * 2 * 128], F16, tag="sn")
            nc.scalar.activation(sn[:], rst[:], SIN, scale=SCALE_2PI)
            # cos(2pi*y) = sin(pi/2 - 2pi*|rs|), same reduction
            ra = trig_pool.tile([128, nb * 2 * 128], F16, tag="ra")
            nc.scalar.activation(ra[:], rst[:],
                                 mybir.ActivationFunctionType.Abs)
            cs = trig_pool.tile([128, nb * 2 * 128], F16, tag="cs")
            nc.scalar.activation(cs[:], ra[:], SIN,
                                 scale=-SCALE_2PI, bias=hpi_sb[:])

            # rotation in [p, t]: x1^T/x2^T are xt chunks dc 0..1 / 2..3
            xv = xtg[:].rearrange("q (b k t) -> q b k t",
                                  b=nb, k=KC, t=128)
            x1 = xv[:, :, 0:2, :]
            x2 = xv[:, :, 2:4, :]
            snv = sn[:].rearrange("q (b c t) -> q b c t", b=nb, c=2, t=128)
            csv = cs[:].rearrange("q (b c t) -> q b c t", b=nb, c=2, t=128)
            wid2 = nb * 2 * 128
            o = out_pool.tile([128, 2 * wid2], F16, tag="o")
            o1 = o[:, 0:wid2].rearrange("q (c b t) -> q b c t",
                                        c=2, b=nb, t=128)
            o2 = o[:, wid2:2 * wid2].rearrange("q (c b t) -> q b c t",
                                               c=2, b=nb, t=128)
            t1 = rot_pool.tile([128, wid2], F16, tag="t1")
            t1v = t1[:].rearrange("q (b c t) -> q b c t", b=nb, c=2, t=128)
            nc.vector.tensor_mul(t1v, x1, csv)
            t2 = rot_pool.tile([128, wid2], F16, tag="t2")
            t2v = t2[:].rearrange("q (b c t) -> q b c t", b=nb, c=2, t=128)
            nc.vector.tensor_mul(t2v, x2, snv)
            nc.vector.tensor_sub(o1, t1v, t2v)
            t3 = rot_pool.tile([128, wid2], F16, tag="t3")
            t3v = t3[:].rearrange("q (b c t) -> q b c t", b=nb, c=2, t=128)
            nc.vector.tensor_mul(t3v, x2, csv)
            t4 = rot_pool.tile([128, wid2], F16, tag="t4")
            t4v = t4[:].rearrange("q (b c t) -> q b c t", b=nb, c=2, t=128)
            nc.vector.tensor_mul(t4v, x1, snv)
            nc.vector.tensor_add(o2, t3v, t4v)

            off = ITEM_OFF[it]
            nc.gpsimd.dma_start(outT[:, off:off + 4 * nb * 128], o[:])

        # remaining w quarters + angle/transpose constants on SP, behind
        # the first xt DMAs (these are only needed by pair-0's back stage)
        def issue_w_rest():
            for q in range(1, 4):
                nc.sync.dma_start(w_sb[:, q * 4 * P:(q + 1) * 4 * P],
                                  w[:, q * 4 * P:(q + 1) * 4 * P])
            nc.sync.dma_start(u_sb[:], u[:])
            nc.sync.dma_start(id_sb[:], ident[:])
            nc.sync.dma_start(afs_sb[:], afs[:])
            nc.sync.dma_start(afm_sb[:], afm[:])

        pend = None  # (it, d16, xtg) awaiting its back stage
        for it in range(len(ITEMS)):
            xtg = issue_in_dmas(it)
            if it == 0:
                issue_w_rest()
            d16 = stage_deltas(it, xtg)
            if pend is not None:
                stage_back(*pend)
            pend = (it, d16, xtg)
        stage_back(*pend)

    nc.compile()
    return nc


_NC_CACHE: dict = {}


def _get_nc():
    if "nc" not in _NC_CACHE:
        _NC_CACHE["nc"] = build_program()
    return _NC_CACHE["nc"]


def prepare_weights(W: np.ndarray, b: np.ndarray):
    inv2pi = 1.0 / (2.0 * np.pi)
    Wt = W.astype(np.float64).T * inv2pi                       # [D, P]
    wh = Wt.astype(np.float16)
    bt = b.astype(np.float64) * inv2pi                         # [P]
    bh = bt.astype(np.float16)
    bl = (bt - bh.astype(np.float64)).astype(np.float16)
    # [D, P] -> [128, KC*P] with d-chunks along the free dim
    w_in = np.ascontiguousarray(
        wh.reshape(KC, 128, P).transpose(1, 0, 2).reshape(128, KC * P))
    # Bases must come from the FULL-precision weights so each 128-step block
    # restarts at the reference-exact angle: the device's fp16-W error then
    # only drifts within one block instead of accumulating across the shard.
    return w_in, bh, bl, Wt, bt


def make_in_maps(x: np.ndarray, W: np.ndarray, b: np.ndarray):
    B, T, _ = x.shape
    w_in, bh, bl, w_eff, b_eff = prepare_weights(W, b)

    u_in = np.triu(np.ones((128, 128), np.float16))
    id_in = np.eye(128, dtype=np.float16)
    afs_in = np.stack([
        np.ones(128, np.float16), np.ones(128, np.float16),
        np.arange(1, 129, dtype=np.float16),
        np.arange(1, 129, dtype=np.float16)])

    # fp64 cumulative angle at every 128-step boundary, per batch (turns)
    nblk = T // BK                                              # 32
    xblk = x.reshape(B, nblk, BK, D).sum(axis=2, dtype=np.float64)
    dblk = xblk @ w_eff + BK * b_eff                            # [B, 32, P]
    bases = np.zeros((B, nblk, P))
    np.cumsum(dblk[:, :-1], axis=1, out=bases[:, 1:])           # exclusive

    in_maps = []
    for c in range(N_CORES):
        bb, hh = c // 2, c % 2
        xs = x[bb, hh * TL:(hh + 1) * TL, :].astype(np.float16)  # [TL, D]
        # xt: [r*128 + dp, (bkl*KC + dc)*128 + tl]
        xt_in = np.ascontiguousarray(
            xs.reshape(NPAIR, 2, BK, KC, 128).transpose(0, 4, 1, 3, 2)
            .reshape(NPAIR * 128, 2 * KC * 128))
        bs = bases[bb, hh * NBK:(hh + 1) * NBK]                 # [NBK, P]
        bs_hi = bs.astype(np.float16)
        bs_lo = (bs - bs_hi.astype(np.float64)).astype(np.float16)
        afm_in = np.stack([
            bs_hi.reshape(NBK * P), bs_lo.reshape(NBK * P),
            np.tile(bh, NBK), np.tile(bl, NBK)])
        in_maps.append({
            "xt": xt_in, "w": w_in, "u": u_in, "ident": id_in,
            "afs": afs_in, "afm": np.ascontiguousarray(afm_in),
        })
    return in_maps


def assemble_output(x: np.ndarray, results) -> np.ndarray:
    B, T, Din = x.shape
    out = np.empty((B, T, Din), np.float32)
    out[:, :, ROT:] = x[:, :, ROT:]
    for c in range(N_CORES):
        bb, hh = c // 2, c % 2
        r = results[c]["outT"]                                # [128, 8192]
        blk = np.empty((TL, ROT), np.float32)
        for it, (bo, nb) in enumerate(ITEMS):
            off = ITEM_OFF[it]
            seg = r[:, off:off + 4 * nb * 128]                # [q, h*c*b*t]
            seg = seg.reshape(128, 2, 2, nb, 128)             # q h c b t
            # rotated[(bo+b)*128 + t, h*256 + c*128 + q]
            seg = seg.transpose(3, 4, 1, 2, 0).reshape(nb * 128, ROT)
            blk[bo * 128:(bo + nb) * 128] = seg
        out[bb, hh * TL:(hh + 1) * TL, :ROT] = blk
    return out


def kernel(x: np.ndarray, W: np.ndarray, b: np.ndarray) -> np.ndarray:
    nc = _get_nc()
    in_maps = make_in_maps(x, W, b)
    res = run_bass_kernel_spmd(nc, in_maps, list(range(N_CORES)))
    return assemble_output(x, res.results)


# revision 18
# speedup vs baseline: 1.0953x; 1.0914x over previous
"""DD-RoPE kernel for 8x TRN2 NeuronCores — pipelined "t-on-partitions" v3.

Reference computation (B=4, T=4096, D=2048, P=256):
    deltas = einsum('btd,pd->btp', x, W) + b     # (B, T, P)
    angles = cumsum(deltas, axis=1)
    out = concat([x1*cos(a) - x2*sin(a), x2*cos(a) + x1*sin(a), x[..., 512:]], -1)

Sharding: 8 shards = 4 batches x 2 T-halves (2048 steps each), data-parallel.
The cumsum is split into independent 128-step blocks via host-computed fp64
block bases (exact cumulative angle at each 128-step boundary) injected on
device through a rank-4 affine matmul, so no cross-core communication and
bounded within-block drift.

v3 design notes (vs the 71us v1/v2):
  - measurement showed steady state was DMA-bound: 1.5 MiB/pair (xt 1 MiB +
    x12 0.25 + out 0.25) at ~360 GB/s = 4.4us/pair vs 3.9us of PE work.
    v3 DELETES the x12 stream entirely: the range-reduced angles are
    transposed on the PE (2 cheap [128,128] transposes per block) and the
    trig + rotation run in [p, t] layout, where x1^T/x2^T are exactly the
    dc=0..3 chunks of the xt tiles already resident for the delta matmuls.
    Per-pair DMA drops to 1.25 MiB = 3.6us < PE 4.2us -> PE-bound.
  - the TRN2 PE p-state ramp (full 2.4 GHz only after 3us of continuous
    execution, reset on idle) is handled by (a) warming the PE with dummy
    matmuls during the DMA prologue and (b) never letting the PE starve:
    the first real matmul is gated on a whole 512 KiB xt block (not a
    fine-grained early chunk that would run dry and reset the ramp).
  - Sin activation table load (1.3us) pulled into the prologue by a dummy
    Sin; x12/out/const DMAs issue from the idle GpSimd sequencer (25ns per
    DMA vs 565ns on SP) so SP only sequences the xt/w stream.
  - tapered tail: the last two blocks run as single-block (256-wide) chains.

Engine budget per pair (cost model): PE 32 MM + 4 angle MM + 4 transposes
= 4.2us; Scalar d16/a_s/sin/cos = 2.8us; DVE rs/|rs|/6 rot = 3.3us;
DMA 1.25 MiB = 3.6us.
"""

import sys

if "/opt/trn_rl_repo" not in sys.path:
    sys.path.insert(0, "/opt/trn_rl_repo")

from contextlib import ExitStack

import numpy as np

import concourse.bacc as bacc
import concourse.bass as bass
import concourse.mybir as mybir
import concourse.tile as tile
from concourse.bass_utils import run_bass_kernel_spmd

F32 = mybir.dt.float32
F16 = mybir.dt.float16
ADD = mybir.AluOpType.add
SUB = mybir.AluOpType.subtract
MULT = mybir.AluOpType.mult
MAX = mybir.AluOpType.max
IDENT = mybir.ActivationFunctionType.Identity
SIN = mybir.ActivationFunctionType.Sin

D = 2048          # input feature dim (contraction)
P = 256           # delta-pairs dim
ROT = 2 * P       # rotated columns (512)
TL = 2048         # time steps per shard
BK = 128          # cumsum block (base injection granularity)
NBK = TL // BK    # blocks per shard (16)
KC = D // 128     # contraction chunks (16)
NPAIR = NBK // 2  # row-pairs in the xt dram layout (8)
N_CORES = 8

# pipeline items: (block offset, blocks in item) — tapered single-block tail
ITEMS = [(0, 2), (2, 2), (4, 2), (6, 2), (8, 2), (10, 2), (12, 2),
         (14, 1), (15, 1)]
# outT column offset per item: 4*nb*128 columns each
ITEM_OFF = []
_off = 0
for _bo, _nb in ITEMS:
    ITEM_OFF.append(_off)
    _off += 4 * _nb * 128
OUT_COLS = _off   # 8192

N_WARM_MM = 10    # dummy matmuls to ramp the PE p-state during the prologue

MAGIC = 12582912.0          # 1.5 * 2**23: fp32 round-to-int magic constant
SCALE_2PI = 6.28310         # slightly < 2*pi so Sin args stay inside [-pi, pi]
HALF_PI = 1.5707964


def build_program() -> bass.Bass:
    nc = bacc.Bacc("TRN2", target_bir_lowering=False, debug=False)

    # x^T tiles: [r*128 + dp, (bkl*KC + dc)*128 + tl] = xs[(2r+bkl)*128+tl,
    #                                                      dc*128 + dp]
    xt = nc.dram_tensor("xt", [NPAIR * 128, 2 * KC * 128], F16,
                        kind="ExternalInput").ap()
    # W, d-chunks along free: [128 d-part, dc*P + p] fp16
    w = nc.dram_tensor("w", [128, KC * P], F16, kind="ExternalInput").ap()
    # upper-triangular ones (u[t, t'] = 1 iff t <= t')
    u = nc.dram_tensor("u", [128, 128], F16, kind="ExternalInput").ap()
    # identity (for PE transposes)
    ident = nc.dram_tensor("ident", [128, 128], F16,
                           kind="ExternalInput").ap()
    # affine stationary: rows [ones, ones, ramp(1..128), ramp]
    afs = nc.dram_tensor("afs", [4, 128], F16, kind="ExternalInput").ap()
    # affine moving: rows [base_hi[bk,p], base_lo, b_hi, b_lo], bk-major
    afm = nc.dram_tensor("afm", [4, NBK * P], F16, kind="ExternalInput").ap()
    # rotated output in [p, t] layout: [q, item_off + (h*2 + c)*nb*128
    #                                       + b*128 + t]
    # (q = p%128, c = p//128, h = rotation half, b = block-in-item, t local)
    outT = nc.dram_tensor("outT", [128, OUT_COLS], F16,
                          kind="ExternalOutput").ap()

    with tile.TileContext(nc) as tc, ExitStack() as ctx:
        const_pool = ctx.enter_context(tc.tile_pool(name="const", bufs=1))
        w_pool = ctx.enter_context(tc.tile_pool(name="w", bufs=1))
        xt_pool = ctx.enter_context(tc.tile_pool(name="xt", bufs=6))
        dp_pool = ctx.enter_context(
            tc.tile_pool(name="dp_psum", bufs=3, space="PSUM"))
        ang_pool = ctx.enter_context(
            tc.tile_pool(name="ang_psum", bufs=2, space="PSUM"))
        rst_pool = ctx.enter_context(
            tc.tile_pool(name="rst_psum", bufs=2, space="PSUM"))
        junk_pool = ctx.enter_context(
            tc.tile_pool(name="junk_psum", bufs=1, space="PSUM"))
        d16_pool = ctx.enter_context(tc.tile_pool(name="d16", bufs=2))
        a32_pool = ctx.enter_context(tc.tile_pool(name="a32", bufs=2))
        trig_pool = ctx.enter_context(tc.tile_pool(name="trig", bufs=2))
        rot_pool = ctx.enter_context(tc.tile_pool(name="rot", bufs=2))
        out_pool = ctx.enter_context(tc.tile_pool(name="out", bufs=2))

        # --- prologue ----------------------------------------------------
        # junk memset + dummy matmuls FIRST, fed from the otherwise-idle
        # DVE queue so the PE p-state ramp starts right after the barrier
        # (a gpsimd-fed memset would chain the PE behind the whole gpsimd
        # prologue through Tile's coalesced per-engine semaphores)
        junk_sb = const_pool.tile([128, 512], F16, tag="junk")
        nc.vector.memset(junk_sb[:], 0.0)
        # zeros for the |rst| abs_max trick (only one DVE operand may be PSUM)
        zero_sb = junk_sb
        junk_ps = junk_pool.tile([128, 512], F32, tag="junkp")
        for _ in range(N_WARM_MM):
            nc.tensor.matmul(junk_ps[:], junk_sb[:, 0:128], junk_sb[:],
                             start=True, stop=True)

        # critical first transfers from the Activation queue (parallel to
        # SP's preamble): first w quarter + xt block 0 (in issue_in_dmas)
        w_sb = w_pool.tile([128, KC * P], F16, tag="w")
        nc.scalar.dma_start(w_sb[:, 0:4 * P], w[:, 0:4 * P])

        u_sb = const_pool.tile([128, 128], F16, tag="u")
        id_sb = const_pool.tile([128, 128], F16, tag="ident")
        afs_sb = const_pool.tile([4, 128], F16, tag="afs")
        afm_sb = const_pool.tile([4, NBK * P], F16, tag="afm")
        magic_sb = const_pool.tile([128, 1], F32, tag="magic")
        nc.gpsimd.memset(magic_sb[:], MAGIC)
        hpi_sb = const_pool.tile([128, 1], F32, tag="hpi")
        nc.gpsimd.memset(hpi_sb[:], HALF_PI)
        # dummy Sin pulls the 1.3us ACT_TABLE_LOAD into the prologue
        warm_sb = const_pool.tile([128, 1], F16, tag="warm")
        nc.gpsimd.memset(warm_sb[:], 0.0)
        warm2_sb = const_pool.tile([128, 1], F16, tag="warm2")
        nc.scalar.activation(warm2_sb[:], warm_sb[:], SIN)

        def issue_in_dmas(it):
            bo, nb = ITEMS[it]
            r, lo = bo // 2, bo % 2
            rows = slice(r * 128, (r + 1) * 128)
            xtg = xt_pool.tile([128, nb * KC * 128], F16, tag="xt")
            if it == 0:
                # one DMA per block from the Activation queue: the first
                # matmul waits for a whole block so the PE never starves
                # mid-block
                nc.scalar.dma_start(xtg[:, 0:KC * 128], xt[rows, 0:KC * 128])
                nc.scalar.dma_start(xtg[:, KC * 128:2 * KC * 128],
                                    xt[rows, KC * 128:2 * KC * 128])
            else:
                xsl = slice(lo * KC * 128, (lo + nb) * KC * 128)
                nc.sync.dma_start(xtg[:], xt[rows, xsl])
            return xtg

        def stage_deltas(it, xtg):
            bo, nb = ITEMS[it]
            wid = nb * P
            dp = dp_pool.tile([128, wid], F32, tag="dp")
            for bkl in range(nb):
                sl = slice(bkl * P, (bkl + 1) * P)
                for dc in range(KC):
                    nc.tensor.matmul(
                        dp[:, sl],
                        xtg[:, (bkl * KC + dc) * 128:(bkl * KC + dc + 1) * 128],
                        w_sb[:, dc * P:(dc + 1) * P],
                        start=(dc == 0), stop=(dc == KC - 1))
            # PSUM->SBUF fp16 downcast on DVE (Scalar is the fuller engine:
            # it already runs a_s, |rst|, sin and cos per item)
            d16 = d16_pool.tile([128, wid], F16, tag="d16")
            nc.vector.tensor_copy(d16[:], dp[:])
            return d16

        def stage_back(it, d16, xtg):
            """Angle matmuls + trig + [p,t] rotation + out DMA for item."""
            bo, nb = ITEMS[it]
            wid = nb * P
            ang = ang_pool.tile([128, wid], F32, tag="ang")
            for bkl in range(nb):
                bk = bo + bkl
                sl = slice(bkl * P, (bkl + 1) * P)
                nc.tensor.matmul(ang[:, sl], u_sb[:], d16[:, sl],
                                 start=True, stop=False)
                nc.tensor.matmul(ang[:, sl], afs_sb[:],
                                 afm_sb[:, bk * P:(bk + 1) * P],
                                 start=False, stop=True)

            # range reduction (turns): rs = y - round(y) in [-0.5, 0.5]
            a_s = a32_pool.tile([128, wid], F32, tag="a_s")
            nc.scalar.activation(a_s[:], ang[:], IDENT,
                                 bias=magic_sb[:], scale=-1.0)
            rs = trig_pool.tile([128, wid], F16, tag="rs")
            nc.vector.scalar_tensor_tensor(rs[:], a_s[:], MAGIC, ang[:],
                                           op0=SUB, op1=ADD)

            # transpose rs to [p, t]: per (block, p-chunk) 128x128 PE
            # transpose; rsT cols = (b, c, t)
            rst = rst_pool.tile([128, nb * 2 * 128], F16, tag="rst")
            for bkl in range(nb):
                for pc in range(2):
                    nc.tensor.transpose(
                        rst[:, (bkl * 2 + pc) * 128:(bkl * 2 + pc + 1) * 128],
                        rs[:, bkl * P + pc * 128:bkl * P + (pc + 1) * 128],
                        id_sb[:])

            sn = trig_pool.tile([128, nb * 2 * 128], F16, tag="sn")
            nc.scalar.activation(sn[:], rst[:], SIN, scale=SCALE_2PI)
            # cos(2pi*y) = sin(pi/2 - 2pi*|rs|), same reduction
            ra = trig_pool.tile([128, nb * 2 * 128], F16, tag="ra")
            nc.scalar.activation(ra[:], rst[:],
                                 mybir.ActivationFunctionType.Abs)
            cs = trig_pool.tile([128, nb * 2 * 128], F16, tag="cs")
            nc.scalar.activation(cs[:], ra[:], SIN,
                                 scale=-SCALE_2PI, bias=hpi_sb[:])

            # rotation in [p, t]: x1^T/x2^T are xt chunks dc 0..1 / 2..3
            xv = xtg[:].rearrange("q (b k t) -> q b k t",
                                  b=nb, k=KC, t=128)
            x1 = xv[:, :, 0:2, :]
            x2 = xv[:, :, 2:4, :]
            snv = sn[:].rearrange("q (b c t) -> q b c t", b=nb, c=2, t=128)
            csv = cs[:].rearrange("q (b c t) -> q b c t", b=nb, c=2, t=128)
            wid2 = nb * 2 * 128
            o = out_pool.tile([128, 2 * wid2], F16, tag="o")
            o1 = o[:, 0:wid2].rearrange("q (c b t) -> q b c t",
                                        c=2, b=nb, t=128)
            o2 = o[:, wid2:2 * wid2].rearrange("q (c b t) -> q b c t",
                                               c=2, b=nb, t=128)
            t1 = rot_pool.tile([128, wid2], F16, tag="t1")
            t1v = t1[:].rearrange("q (b c t) -> q b c t", b=nb, c=2, t=128)
            nc.vector.tensor_mul(t1v, x1, csv)
            t2 = rot_pool.tile([128, wid2], F16, tag="t2")
            t2v = t2[:].rearrange("q (b c t) -> q b c t", b=nb, c=2, t=128)
            nc.vector.tensor_mul(t2v, x2, snv)
            nc.vector.tensor_sub(o1, t1v, t2v)
            t3 = rot_pool.tile([128, wid2], F16, tag="t3")
            t3v = t3[:].rearrange("q (b c t) -> q b c t", b=nb, c=2, t=128)
            nc.vector.tensor_mul(t3v, x2, csv)
            t4 = rot_pool.tile([128, wid2], F16, tag="t4")
            t4v = t4[:].rearrange("q (b c t) -> q b c t", b=nb, c=2, t=128)
            nc.vector.tensor_mul(t4v, x1, snv)
            nc.vector.tensor_add(o2, t3v, t4v)

            off = ITEM_OFF[it]
            nc.gpsimd.dma_start(outT[:, off:off + 4 * nb * 128], o[:])

        # remaining w quarters + angle/transpose constants on SP, behind
        # the first xt DMAs (these are only needed by pair-0's back stage)
        def issue_w_rest():
            for q in range(1, 4):
                nc.sync.dma_start(w_sb[:, q * 4 * P:(q + 1) * 4 * P],
                                  w[:, q * 4 * P:(q + 1) * 4 * P])
            nc.sync.dma_start(u_sb[:], u[:])
            nc.sync.dma_start(id_sb[:], ident[:])
            nc.sync.dma_start(afs_sb[:], afs[:])
            nc.sync.dma_start(afm_sb[:], afm[:])

        pend = None  # (it, d16, xtg) awaiting its back stage
        for it in range(len(ITEMS)):
            xtg = issue_in_dmas(it)
            if it == 0:
                issue_w_rest()
            d16 = stage_deltas(it, xtg)
            if pend is not None:
                stage_back(*pend)
            pend = (it, d16, xtg)
        stage_back(*pend)

    nc.compile()
    return nc


_NC_CACHE: dict = {}


def _get_nc():
    if "nc" not in _NC_CACHE:
        _NC_CACHE["nc"] = build_program()
    return _NC_CACHE["nc"]


def prepare_weights(W: np.ndarray, b: np.ndarray):
    inv2pi = 1.0 / (2.0 * np.pi)
    Wt = W.astype(np.float64).T * inv2pi                       # [D, P]
    wh = Wt.astype(np.float16)
    bt = b.astype(np.float64) * inv2pi                         # [P]
    bh = bt.astype(np.float16)
    bl = (bt - bh.astype(np.float64)).astype(np.float16)
    # [D, P] -> [128, KC*P] with d-chunks along the free dim
    w_in = np.ascontiguousarray(
        wh.reshape(KC, 128, P).transpose(1, 0, 2).reshape(128, KC * P))
    # Bases must come from the FULL-precision weights so each 128-step block
    # restarts at the reference-exact angle: the device's fp16-W error then
    # only drifts within one block instead of accumulating across the shard.
    return w_in, bh, bl, Wt, bt


def make_in_maps(x: np.ndarray, W: np.ndarray, b: np.ndarray):
    B, T, _ = x.shape
    w_in, bh, bl, w_eff, b_eff = prepare_weights(W, b)

    u_in = np.triu(np.ones((128, 128), np.float16))
    id_in = np.eye(128, dtype=np.float16)
    afs_in = np.stack([
        np.ones(128, np.float16), np.ones(128, np.float16),
        np.arange(1, 129, dtype=np.float16),
        np.arange(1, 129, dtype=np.float16)])

    # fp64 cumulative angle at every 128-step boundary, per batch (turns)
    nblk = T // BK                                              # 32
    xblk = x.reshape(B, nblk, BK, D).sum(axis=2, dtype=np.float64)
    dblk = xblk @ w_eff + BK * b_eff                            # [B, 32, P]
    bases = np.zeros((B, nblk, P))
    np.cumsum(dblk[:, :-1], axis=1, out=bases[:, 1:])           # exclusive

    in_maps = []
    for c in range(N_CORES):
        bb, hh = c // 2, c % 2
        xs = x[bb, hh * TL:(hh + 1) * TL, :].astype(np.float16)  # [TL, D]
        # xt: [r*128 + dp, (bkl*KC + dc)*128 + tl]
        xt_in = np.ascontiguousarray(
            xs.reshape(NPAIR, 2, BK, KC, 128).transpose(0, 4, 1, 3, 2)
            .reshape(NPAIR * 128, 2 * KC * 128))
        bs = bases[bb, hh * NBK:(hh + 1) * NBK]                 # [NBK, P]
        bs_hi = bs.astype(np.float16)
        bs_lo = (bs - bs_hi.astype(np.float64)).astype(np.float16)
        afm_in = np.stack([
            bs_hi.reshape(NBK * P), bs_lo.reshape(NBK * P),
            np.tile(bh, NBK), np.tile(bl, NBK)])
        in_maps.append({
            "xt": xt_in, "w": w_in, "u": u_in, "ident": id_in,
            "afs": afs_in, "afm": np.ascontiguousarray(afm_in),
        })
    return in_maps


def assemble_output(x: np.ndarray, results) -> np.ndarray:
    B, T, Din = x.shape
    out = np.empty((B, T, Din), np.float32)
    out[:, :, ROT:] = x[:, :, ROT:]
    for c in range(N_CORES):
        bb, hh = c // 2, c % 2
        r = results[c]["outT"]                                # [128, 8192]
        blk = np.empty((TL, ROT), np.float32)
        for it, (bo, nb) in enumerate(ITEMS):
            off = ITEM_OFF[it]
            seg = r[:, off:off + 4 * nb * 128]                # [q, h*c*b*t]
            seg = seg.reshape(128, 2, 2, nb, 128)             # q h c b t
            # rotated[(bo+b)*128 + t, h*256 + c*128 + q]
            seg = seg.transpose(3, 4, 1, 2, 0).reshape(nb * 128, ROT)
            blk[bo * 128:(bo + nb) * 128] = seg
        out[bb, hh * TL:(hh + 1) * TL, :ROT] = blk
    return out


def kernel(x: np.ndarray, W: np.ndarray, b: np.ndarray) -> np.ndarray:
    nc = _get_nc()
    in_maps = make_in_maps(x, W, b)
    res = run_bass_kernel_spmd(nc, in_maps, list(range(N_CORES)))
    return assemble_output(x, res.results)


# revision 21
# speedup vs baseline: 1.1688x; 1.0671x over previous
"""DD-RoPE kernel for 8x TRN2 NeuronCores — pipelined "t-on-partitions" v3.

Reference computation (B=4, T=4096, D=2048, P=256):
    deltas = einsum('btd,pd->btp', x, W) + b     # (B, T, P)
    angles = cumsum(deltas, axis=1)
    out = concat([x1*cos(a) - x2*sin(a), x2*cos(a) + x1*sin(a), x[..., 512:]], -1)

Sharding: 8 shards = 4 batches x 2 T-halves (2048 steps each), data-parallel.
The cumsum is split into independent 128-step blocks via host-computed fp64
block bases (exact cumulative angle at each 128-step boundary) injected on
device through a rank-4 affine matmul, so no cross-core communication and
bounded within-block drift.

v3 design notes (vs the 71us v1/v2):
  - measurement showed steady state was DMA-bound: 1.5 MiB/pair (xt 1 MiB +
    x12 0.25 + out 0.25) at ~360 GB/s = 4.4us/pair vs 3.9us of PE work.
    v3 DELETES the x12 stream entirely: the range-reduced angles are
    transposed on the PE (2 cheap [128,128] transposes per block) and the
    trig + rotation run in [p, t] layout, where x1^T/x2^T are exactly the
    dc=0..3 chunks of the xt tiles already resident for the delta matmuls.
    Per-pair DMA drops to 1.25 MiB = 3.6us < PE 4.2us -> PE-bound.
  - the TRN2 PE p-state ramp (full 2.4 GHz only after 3us of continuous
    execution, reset on idle) is handled by (a) warming the PE with dummy
    matmuls during the DMA prologue and (b) never letting the PE starve:
    the first real matmul is gated on a whole 512 KiB xt block (not a
    fine-grained early chunk that would run dry and reset the ramp).
  - Sin activation table load (1.3us) pulled into the prologue by a dummy
    Sin; x12/out/const DMAs issue from the idle GpSimd sequencer (25ns per
    DMA vs 565ns on SP) so SP only sequences the xt/w stream.
  - tapered tail: the last two blocks run as single-block (256-wide) chains.

Engine budget per pair (cost model): PE 32 MM + 4 angle MM + 4 transposes
= 4.2us; Scalar d16/a_s/sin/cos = 2.8us; DVE rs/|rs|/6 rot = 3.3us;
DMA 1.25 MiB = 3.6us.
"""

import sys

if "/opt/trn_rl_repo" not in sys.path:
    sys.path.insert(0, "/opt/trn_rl_repo")

from contextlib import ExitStack

import numpy as np

import concourse.bacc as bacc
import concourse.bass as bass
import concourse.mybir as mybir
import concourse.tile as tile
from concourse.bass_utils import run_bass_kernel_spmd

F32 = mybir.dt.float32
F16 = mybir.dt.float16
ADD = mybir.AluOpType.add
SUB = mybir.AluOpType.subtract
MULT = mybir.AluOpType.mult
MAX = mybir.AluOpType.max
IDENT = mybir.ActivationFunctionType.Identity
SIN = mybir.ActivationFunctionType.Sin

D = 2048          # input feature dim (contraction)
P = 256           # delta-pairs dim
ROT = 2 * P       # rotated columns (512)
TL = 2048         # time steps per shard
BK = 128          # cumsum block (base injection granularity)
NBK = TL // BK    # blocks per shard (16)
KC = D // 128     # contraction chunks (16)
NPAIR = NBK // 2  # row-pairs in the xt dram layout (8)
N_CORES = 8

# pipeline items: (block offset, blocks in item) — tapered single-block tail
ITEMS = [(0, 2), (2, 2), (4, 2), (6, 2), (8, 2), (10, 2), (12, 2),
         (14, 1), (15, 1)]
# outT column offset per item: 4*nb*128 columns each
ITEM_OFF = []
_off = 0
for _bo, _nb in ITEMS:
    ITEM_OFF.append(_off)
    _off += 4 * _nb * 128
OUT_COLS = _off   # 8192

N_WARM_MM = 10    # dummy matmuls to ramp the PE p-state during the prologue

MAGIC = 12582912.0          # 1.5 * 2**23: fp32 round-to-int magic constant
SCALE_2PI = 6.28310         # slightly < 2*pi so Sin args stay inside [-pi, pi]
HALF_PI = 1.5707964


def build_program() -> bass.Bass:
    nc = bacc.Bacc("TRN2", target_bir_lowering=False, debug=False)

    # x^T tiles: [r*128 + dp, (bkl*KC + dc)*128 + tl] = xs[(2r+bkl)*128+tl,
    #                                                      dc*128 + dp]
    xt = nc.dram_tensor("xt", [NPAIR * 128, 2 * KC * 128], F16,
                        kind="ExternalInput").ap()
    # W, d-chunks along free: [128 d-part, dc*P + p] fp16
    w = nc.dram_tensor("w", [128, KC * P], F16, kind="ExternalInput").ap()
    # upper-triangular ones (u[t, t'] = 1 iff t <= t')
    u = nc.dram_tensor("u", [128, 128], F16, kind="ExternalInput").ap()
    # identity (for PE transposes)
    ident = nc.dram_tensor("ident", [128, 128], F16,
                           kind="ExternalInput").ap()
    # affine stationary: rows [ones, ones, ramp(1..128), ramp]
    afs = nc.dram_tensor("afs", [4, 128], F16, kind="ExternalInput").ap()
    # affine moving: rows [base_hi[bk,p], base_lo, b_hi, b_lo], bk-major
    afm = nc.dram_tensor("afm", [4, NBK * P], F16, kind="ExternalInput").ap()
    # rotated output in [p, t] layout: [q, item_off + (h*2 + c)*nb*128
    #                                       + b*128 + t]
    # (q = p%128, c = p//128, h = rotation half, b = block-in-item, t local)
    outT = nc.dram_tensor("outT", [128, OUT_COLS], F16,
                          kind="ExternalOutput").ap()

    with tile.TileContext(nc) as tc, ExitStack() as ctx:
        const_pool = ctx.enter_context(tc.tile_pool(name="const", bufs=1))
        w_pool = ctx.enter_context(tc.tile_pool(name="w", bufs=1))
        xt_pool = ctx.enter_context(tc.tile_pool(name="xt", bufs=6))
        dp_pool = ctx.enter_context(
            tc.tile_pool(name="dp_psum", bufs=3, space="PSUM"))
        ang_pool = ctx.enter_context(
            tc.tile_pool(name="ang_psum", bufs=2, space="PSUM"))
        rst_pool = ctx.enter_context(
            tc.tile_pool(name="rst_psum", bufs=2, space="PSUM"))
        junk_pool = ctx.enter_context(
            tc.tile_pool(name="junk_psum", bufs=1, space="PSUM"))
        d16_pool = ctx.enter_context(tc.tile_pool(name="d16", bufs=3))
        a32_pool = ctx.enter_context(tc.tile_pool(name="a32", bufs=3))
        trig_pool = ctx.enter_context(tc.tile_pool(name="trig", bufs=3))
        rot_pool = ctx.enter_context(tc.tile_pool(name="rot", bufs=2))
        out_pool = ctx.enter_context(tc.tile_pool(name="out", bufs=3))

        # --- prologue ----------------------------------------------------
        # junk memset + dummy matmuls FIRST, fed from the otherwise-idle
        # DVE queue so the PE p-state ramp starts right after the barrier
        # (a gpsimd-fed memset would chain the PE behind the whole gpsimd
        # prologue through Tile's coalesced per-engine semaphores)
        junk_sb = const_pool.tile([128, 512], F16, tag="junk")
        nc.vector.memset(junk_sb[:], 0.0)
        # zeros for the |rst| abs_max trick (only one DVE operand may be PSUM)
        zero_sb = junk_sb
        junk_ps = junk_pool.tile([128, 512], F32, tag="junkp")
        for _ in range(N_WARM_MM):
            nc.tensor.matmul(junk_ps[:], junk_sb[:, 0:128], junk_sb[:],
                             start=True, stop=True)

        # critical first transfers from the Activation queue (parallel to
        # SP's preamble): first w quarter + xt block 0 (in issue_in_dmas)
        w_sb = w_pool.tile([128, KC * P], F16, tag="w")
        nc.scalar.dma_start(w_sb[:, 0:4 * P], w[:, 0:4 * P])

        u_sb = const_pool.tile([128, 128], F16, tag="u")
        id_sb = const_pool.tile([128, 128], F16, tag="ident")
        afs_sb = const_pool.tile([4, 128], F16, tag="afs")
        afm_sb = const_pool.tile([4, NBK * P], F16, tag="afm")
        magic_sb = const_pool.tile([128, 1], F32, tag="magic")
        nc.gpsimd.memset(magic_sb[:], MAGIC)
        hpi_sb = const_pool.tile([128, 1], F32, tag="hpi")
        nc.gpsimd.memset(hpi_sb[:], HALF_PI)
        # dummy Sin pulls the 1.3us ACT_TABLE_LOAD into the prologue
        warm_sb = const_pool.tile([128, 1], F16, tag="warm")
        nc.gpsimd.memset(warm_sb[:], 0.0)
        warm2_sb = const_pool.tile([128, 1], F16, tag="warm2")
        nc.scalar.activation(warm2_sb[:], warm_sb[:], SIN)

        def issue_in_dmas(it):
            bo, nb = ITEMS[it]
            r, lo = bo // 2, bo % 2
            rows = slice(r * 128, (r + 1) * 128)
            xtg = xt_pool.tile([128, nb * KC * 128], F16, tag="xt")
            if it == 0:
                # one DMA per block from the Activation queue: the first
                # matmul waits for a whole block so the PE never starves
                # mid-block
                nc.scalar.dma_start(xtg[:, 0:KC * 128], xt[rows, 0:KC * 128])
                nc.scalar.dma_start(xtg[:, KC * 128:2 * KC * 128],
                                    xt[rows, KC * 128:2 * KC * 128])
            else:
                xsl = slice(lo * KC * 128, (lo + nb) * KC * 128)
                nc.sync.dma_start(xtg[:], xt[rows, xsl])
            return xtg

        def stage_deltas(it, xtg):
            bo, nb = ITEMS[it]
            wid = nb * P
            dp = dp_pool.tile([128, wid], F32, tag="dp")
            for bkl in range(nb):
                sl = slice(bkl * P, (bkl + 1) * P)
                for dc in range(KC):
                    nc.tensor.matmul(
                        dp[:, sl],
                        xtg[:, (bkl * KC + dc) * 128:(bkl * KC + dc + 1) * 128],
                        w_sb[:, dc * P:(dc + 1) * P],
                        start=(dc == 0), stop=(dc == KC - 1))
            # PSUM->SBUF fp16 downcast on DVE (Scalar is the fuller engine:
            # it already runs a_s, |rst|, sin and cos per item)
            d16 = d16_pool.tile([128, wid], F16, tag="d16")
            nc.vector.tensor_copy(d16[:], dp[:])
            return d16

        def stage_back(it, d16, xtg):
            """Angle matmuls + trig + [p,t] rotation + out DMA for item."""
            bo, nb = ITEMS[it]
            wid = nb * P
            ang = ang_pool.tile([128, wid], F32, tag="ang")
            for bkl in range(nb):
                bk = bo + bkl
                sl = slice(bkl * P, (bkl + 1) * P)
                nc.tensor.matmul(ang[:, sl], u_sb[:], d16[:, sl],
                                 start=True, stop=False)
                nc.tensor.matmul(ang[:, sl], afs_sb[:],
                                 afm_sb[:, bk * P:(bk + 1) * P],
                                 start=False, stop=True)

            # range reduction (turns): rs = y - round(y) in [-0.5, 0.5]
            a_s = a32_pool.tile([128, wid], F32, tag="a_s")
            nc.scalar.activation(a_s[:], ang[:], IDENT,
                                 bias=magic_sb[:], scale=-1.0)
            rs = trig_pool.tile([128, wid], F16, tag="rs")
            nc.vector.scalar_tensor_tensor(rs[:], a_s[:], MAGIC, ang[:],
                                           op0=SUB, op1=ADD)

            # transpose rs to [p, t]: per (block, p-chunk) 128x128 PE
            # transpose; rsT cols = (b, c, t)
            rst = rst_pool.tile([128, nb * 2 * 128], F16, tag="rst")
            for bkl in range(nb):
                for pc in range(2):
                    nc.tensor.transpose(
                        rst[:, (bkl * 2 + pc) * 128:(bkl * 2 + pc + 1) * 128],
                        rs[:, bkl * P + pc * 128:bkl * P + (pc + 1) * 128],
                        id_sb[:])

            sn = trig_pool.tile([128, nb * 2 * 128], F16, tag="sn")
            nc.scalar.activation(sn[:], rst[:], SIN, scale=SCALE_2PI)
            # cos(2pi*y) = sin(pi/2 - 2pi*|rs|), same reduction
            ra = trig_pool.tile([128, nb * 2 * 128], F16, tag="ra")
            nc.scalar.activation(ra[:], rst[:],
                                 mybir.ActivationFunctionType.Abs)
            cs = trig_pool.tile([128, nb * 2 * 128], F16, tag="cs")
            nc.scalar.activation(cs[:], ra[:], SIN,
                                 scale=-SCALE_2PI, bias=hpi_sb[:])

            # rotation in [p, t]: x1^T/x2^T are xt chunks dc 0..1 / 2..3
            xv = xtg[:].rearrange("q (b k t) -> q b k t",
                                  b=nb, k=KC, t=128)
            x1 = xv[:, :, 0:2, :]
            x2 = xv[:, :, 2:4, :]
            snv = sn[:].rearrange("q (b c t) -> q b c t", b=nb, c=2, t=128)
            csv = cs[:].rearrange("q (b c t) -> q b c t", b=nb, c=2, t=128)
            wid2 = nb * 2 * 128
            o = out_pool.tile([128, 2 * wid2], F16, tag="o")
            o1 = o[:, 0:wid2].rearrange("q (c b t) -> q b c t",
                                        c=2, b=nb, t=128)
            o2 = o[:, wid2:2 * wid2].rearrange("q (c b t) -> q b c t",
                                               c=2, b=nb, t=128)
            t1 = rot_pool.tile([128, wid2], F16, tag="t1")
            t1v = t1[:].rearrange("q (b c t) -> q b c t", b=nb, c=2, t=128)
            nc.vector.tensor_mul(t1v, x1, csv)
            t2 = rot_pool.tile([128, wid2], F16, tag="t2")
            t2v = t2[:].rearrange("q (b c t) -> q b c t", b=nb, c=2, t=128)
            nc.vector.tensor_mul(t2v, x2, snv)
            nc.vector.tensor_sub(o1, t1v, t2v)
            t3 = rot_pool.tile([128, wid2], F16, tag="t3")
            t3v = t3[:].rearrange("q (b c t) -> q b c t", b=nb, c=2, t=128)
            nc.vector.tensor_mul(t3v, x2, csv)
            t4 = rot_pool.tile([128, wid2], F16, tag="t4")
            t4v = t4[:].rearrange("q (b c t) -> q b c t", b=nb, c=2, t=128)
            nc.vector.tensor_mul(t4v, x1, snv)
            nc.vector.tensor_add(o2, t3v, t4v)

            off = ITEM_OFF[it]
            nc.gpsimd.dma_start(outT[:, off:off + 4 * nb * 128], o[:])

        # remaining w quarters + angle/transpose constants on SP, behind
        # the first xt DMAs (these are only needed by pair-0's back stage)
        def issue_w_rest():
            for q in range(1, 4):
                nc.sync.dma_start(w_sb[:, q * 4 * P:(q + 1) * 4 * P],
                                  w[:, q * 4 * P:(q + 1) * 4 * P])
            nc.sync.dma_start(u_sb[:], u[:])
            nc.sync.dma_start(id_sb[:], ident[:])
            nc.sync.dma_start(afs_sb[:], afs[:])
            nc.sync.dma_start(afm_sb[:], afm[:])

        # back(it-1) is emitted BEFORE deltas(it): the list scheduler then
        # prefers the previous item's angle-matmuls/transposes over the next
        # delta batch the moment their inputs are ready, keeping the
        # back-stage pipeline short instead of queueing it behind 32 delta
        # matmuls
        pend = None  # (it, d16, xtg) awaiting its back stage
        for it in range(len(ITEMS)):
            xtg = issue_in_dmas(it)
            if it == 0:
                issue_w_rest()
            if pend is not None:
                stage_back(*pend)
            d16 = stage_deltas(it, xtg)
            pend = (it, d16, xtg)
        stage_back(*pend)

    nc.compile()
    return nc


_NC_CACHE: dict = {}


def _get_nc():
    if "nc" not in _NC_CACHE:
        _NC_CACHE["nc"] = build_program()
    return _NC_CACHE["nc"]


def prepare_weights(W: np.ndarray, b: np.ndarray):
    inv2pi = 1.0 / (2.0 * np.pi)
    Wt = W.astype(np.float64).T * inv2pi                       # [D, P]
    wh = Wt.astype(np.float16)
    bt = b.astype(np.float64) * inv2pi                         # [P]
    bh = bt.astype(np.float16)
    bl = (bt - bh.astype(np.float64)).astype(np.float16)
    # [D, P] -> [128, KC*P] with d-chunks along the free dim
    w_in = np.ascontiguousarray(
        wh.reshape(KC, 128, P).transpose(1, 0, 2).reshape(128, KC * P))
    # Bases must come from the FULL-precision weights so each 128-step block
    # restarts at the reference-exact angle: the device's fp16-W error then
    # only drifts within one block instead of accumulating across the shard.
    return w_in, bh, bl, Wt, bt


def make_in_maps(x: np.ndarray, W: np.ndarray, b: np.ndarray):
    B, T, _ = x.shape
    w_in, bh, bl, w_eff, b_eff = prepare_weights(W, b)

    u_in = np.triu(np.ones((128, 128), np.float16))
    id_in = np.eye(128, dtype=np.float16)
    afs_in = np.stack([
        np.ones(128, np.float16), np.ones(128, np.float16),
        np.arange(1, 129, dtype=np.float16),
        np.arange(1, 129, dtype=np.float16)])

    # fp64 cumulative angle at every 128-step boundary, per batch (turns)
    nblk = T // BK                                              # 32
    xblk = x.reshape(B, nblk, BK, D).sum(axis=2, dtype=np.float64)
    dblk = xblk @ w_eff + BK * b_eff                            # [B, 32, P]
    bases = np.zeros((B, nblk, P))
    np.cumsum(dblk[:, :-1], axis=1, out=bases[:, 1:])           # exclusive

    in_maps = []
    for c in range(N_CORES):
        bb, hh = c // 2, c % 2
        xs = x[bb, hh * TL:(hh + 1) * TL, :].astype(np.float16)  # [TL, D]
        # xt: [r*128 + dp, (bkl*KC + dc)*128 + tl]
        xt_in = np.ascontiguousarray(
            xs.reshape(NPAIR, 2, BK, KC, 128).transpose(0, 4, 1, 3, 2)
            .reshape(NPAIR * 128, 2 * KC * 128))
        bs = bases[bb, hh * NBK:(hh + 1) * NBK]                 # [NBK, P]
        bs_hi = bs.astype(np.float16)
        bs_lo = (bs - bs_hi.astype(np.float64)).astype(np.float16)
        afm_in = np.stack([
            bs_hi.reshape(NBK * P), bs_lo.reshape(NBK * P),
            np.tile(bh, NBK), np.tile(bl, NBK)])
        in_maps.append({
            "xt": xt_in, "w": w_in, "u": u_in, "ident": id_in,
            "afs": afs_in, "afm": np.ascontiguousarray(afm_in),
        })
    return in_maps


def assemble_output(x: np.ndarray, results) -> np.ndarray:
    B, T, Din = x.shape
    out = np.empty((B, T, Din), np.float32)
    out[:, :, ROT:] = x[:, :, ROT:]
    for c in range(N_CORES):
        bb, hh = c // 2, c % 2
        r = results[c]["outT"]                                # [128, 8192]
        blk = np.empty((TL, ROT), np.float32)
        for it, (bo, nb) in enumerate(ITEMS):
            off = ITEM_OFF[it]
            seg = r[:, off:off + 4 * nb * 128]                # [q, h*c*b*t]
            seg = seg.reshape(128, 2, 2, nb, 128)             # q h c b t
            # rotated[(bo+b)*128 + t, h*256 + c*128 + q]
            seg = seg.transpose(3, 4, 1, 2, 0).reshape(nb * 128, ROT)
            blk[bo * 128:(bo + nb) * 128] = seg
        out[bb, hh * TL:(hh + 1) * TL, :ROT] = blk
    return out


def kernel(x: np.ndarray, W: np.ndarray, b: np.ndarray) -> np.ndarray:
    nc = _get_nc()
    in_maps = make_in_maps(x, W, b)
    res = run_bass_kernel_spmd(nc, in_maps, list(range(N_CORES)))
    return assemble_output(x, res.results)


# revision 22
# speedup vs baseline: 1.1715x; 1.0024x over previous
"""DD-RoPE kernel for 8x TRN2 NeuronCores — pipelined "t-on-partitions" v3.

Reference computation (B=4, T=4096, D=2048, P=256):
    deltas = einsum('btd,pd->btp', x, W) + b     # (B, T, P)
    angles = cumsum(deltas, axis=1)
    out = concat([x1*cos(a) - x2*sin(a), x2*cos(a) + x1*sin(a), x[..., 512:]], -1)

Sharding: 8 shards = 4 batches x 2 T-halves (2048 steps each), data-parallel.
The cumsum is split into independent 128-step blocks via host-computed fp64
block bases (exact cumulative angle at each 128-step boundary) injected on
device through a rank-4 affine matmul, so no cross-core communication and
bounded within-block drift.

v3 design notes (vs the 71us v1/v2):
  - measurement showed steady state was DMA-bound: 1.5 MiB/pair (xt 1 MiB +
    x12 0.25 + out 0.25) at ~360 GB/s = 4.4us/pair vs 3.9us of PE work.
    v3 DELETES the x12 stream entirely: the range-reduced angles are
    transposed on the PE (2 cheap [128,128] transposes per block) and the
    trig + rotation run in [p, t] layout, where x1^T/x2^T are exactly the
    dc=0..3 chunks of the xt tiles already resident for the delta matmuls.
    Per-pair DMA drops to 1.25 MiB = 3.6us < PE 4.2us -> PE-bound.
  - the TRN2 PE p-state ramp (full 2.4 GHz only after 3us of continuous
    execution, reset on idle) is handled by (a) warming the PE with dummy
    matmuls during the DMA prologue and (b) never letting the PE starve:
    the first real matmul is gated on a whole 512 KiB xt block (not a
    fine-grained early chunk that would run dry and reset the ramp).
  - Sin activation table load (1.3us) pulled into the prologue by a dummy
    Sin; x12/out/const DMAs issue from the idle GpSimd sequencer (25ns per
    DMA vs 565ns on SP) so SP only sequences the xt/w stream.
  - tapered tail: the last two blocks run as single-block (256-wide) chains.

Engine budget per pair (cost model): PE 32 MM + 4 angle MM + 4 transposes
= 4.2us; Scalar d16/a_s/sin/cos = 2.8us; DVE rs/|rs|/6 rot = 3.3us;
DMA 1.25 MiB = 3.6us.
"""

import sys

if "/opt/trn_rl_repo" not in sys.path:
    sys.path.insert(0, "/opt/trn_rl_repo")

from contextlib import ExitStack

import numpy as np

import concourse.bacc as bacc
import concourse.bass as bass
import concourse.mybir as mybir
import concourse.tile as tile
from concourse.bass_utils import run_bass_kernel_spmd

F32 = mybir.dt.float32
F16 = mybir.dt.float16
ADD = mybir.AluOpType.add
SUB = mybir.AluOpType.subtract
MULT = mybir.AluOpType.mult
MAX = mybir.AluOpType.max
IDENT = mybir.ActivationFunctionType.Identity
SIN = mybir.ActivationFunctionType.Sin

D = 2048          # input feature dim (contraction)
P = 256           # delta-pairs dim
ROT = 2 * P       # rotated columns (512)
TL = 2048         # time steps per shard
BK = 128          # cumsum block (base injection granularity)
NBK = TL // BK    # blocks per shard (16)
KC = D // 128     # contraction chunks (16)
NPAIR = NBK // 2  # row-pairs in the xt dram layout (8)
N_CORES = 8

# pipeline items: (block offset, blocks in item) — tapered single-block tail
ITEMS = [(0, 2), (2, 2), (4, 2), (6, 2), (8, 2), (10, 2), (12, 2),
         (14, 1), (15, 1)]
# outT column offset per item: 4*nb*128 columns each
ITEM_OFF = []
_off = 0
for _bo, _nb in ITEMS:
    ITEM_OFF.append(_off)
    _off += 4 * _nb * 128
OUT_COLS = _off   # 8192

N_WARM_MM = 10    # dummy matmuls to ramp the PE p-state during the prologue

MAGIC = 12582912.0          # 1.5 * 2**23: fp32 round-to-int magic constant
SCALE_2PI = 6.28310         # slightly < 2*pi so Sin args stay inside [-pi, pi]
HALF_PI = 1.5707964


def build_program() -> bass.Bass:
    nc = bacc.Bacc("TRN2", target_bir_lowering=False, debug=False)

    # x^T tiles: [r*128 + dp, (bkl*KC + dc)*128 + tl] = xs[(2r+bkl)*128+tl,
    #                                                      dc*128 + dp]
    xt = nc.dram_tensor("xt", [NPAIR * 128, 2 * KC * 128], F16,
                        kind="ExternalInput").ap()
    # W, d-chunks along free: [128 d-part, dc*P + p] fp16
    w = nc.dram_tensor("w", [128, KC * P], F16, kind="ExternalInput").ap()
    # upper-triangular ones (u[t, t'] = 1 iff t <= t')
    u = nc.dram_tensor("u", [128, 128], F16, kind="ExternalInput").ap()
    # identity (for PE transposes)
    ident = nc.dram_tensor("ident", [128, 128], F16,
                           kind="ExternalInput").ap()
    # affine stationary: rows [ones, ones, ramp(1..128), ramp]
    afs = nc.dram_tensor("afs", [4, 128], F16, kind="ExternalInput").ap()
    # affine moving: rows [base_hi[bk,p], base_lo, b_hi, b_lo], bk-major
    afm = nc.dram_tensor("afm", [4, NBK * P], F16, kind="ExternalInput").ap()
    # rotated output in [p, t] layout: [q, item_off + (h*2 + c)*nb*128
    #                                       + b*128 + t]
    # (q = p%128, c = p//128, h = rotation half, b = block-in-item, t local)
    outT = nc.dram_tensor("outT", [128, OUT_COLS], F16,
                          kind="ExternalOutput").ap()

    with tile.TileContext(nc) as tc, ExitStack() as ctx:
        const_pool = ctx.enter_context(tc.tile_pool(name="const", bufs=1))
        w_pool = ctx.enter_context(tc.tile_pool(name="w", bufs=1))
        xt_pool = ctx.enter_context(tc.tile_pool(name="xt", bufs=6))
        dp_pool = ctx.enter_context(
            tc.tile_pool(name="dp_psum", bufs=3, space="PSUM"))
        ang_pool = ctx.enter_context(
            tc.tile_pool(name="ang_psum", bufs=2, space="PSUM"))
        rst_pool = ctx.enter_context(
            tc.tile_pool(name="rst_psum", bufs=2, space="PSUM"))
        junk_pool = ctx.enter_context(
            tc.tile_pool(name="junk_psum", bufs=1, space="PSUM"))
        d16_pool = ctx.enter_context(tc.tile_pool(name="d16", bufs=3))
        a32_pool = ctx.enter_context(tc.tile_pool(name="a32", bufs=3))
        trig_pool = ctx.enter_context(tc.tile_pool(name="trig", bufs=3))
        rot_pool = ctx.enter_context(tc.tile_pool(name="rot", bufs=2))
        out_pool = ctx.enter_context(tc.tile_pool(name="out", bufs=3))

        # --- prologue ----------------------------------------------------
        # junk memset + dummy matmuls FIRST, fed from the otherwise-idle
        # DVE queue so the PE p-state ramp starts right after the barrier
        # (a gpsimd-fed memset would chain the PE behind the whole gpsimd
        # prologue through Tile's coalesced per-engine semaphores)
        junk_sb = const_pool.tile([128, 512], F16, tag="junk")
        nc.vector.memset(junk_sb[:], 0.0)
        # zeros for the |rst| abs_max trick (only one DVE operand may be PSUM)
        zero_sb = junk_sb
        junk_ps = junk_pool.tile([128, 512], F32, tag="junkp")
        for _ in range(N_WARM_MM):
            nc.tensor.matmul(junk_ps[:], junk_sb[:, 0:128], junk_sb[:],
                             start=True, stop=True)

        # critical first transfers from the Activation queue (parallel to
        # SP's preamble): first w quarter + xt block 0 (in issue_in_dmas)
        w_sb = w_pool.tile([128, KC * P], F16, tag="w")
        nc.scalar.dma_start(w_sb[:, 0:4 * P], w[:, 0:4 * P])

        u_sb = const_pool.tile([128, 128], F16, tag="u")
        id_sb = const_pool.tile([128, 128], F16, tag="ident")
        afs_sb = const_pool.tile([4, 128], F16, tag="afs")
        afm_sb = const_pool.tile([4, NBK * P], F16, tag="afm")
        magic_sb = const_pool.tile([128, 1], F32, tag="magic")
        nc.gpsimd.memset(magic_sb[:], MAGIC)
        hpi_sb = const_pool.tile([128, 1], F32, tag="hpi")
        nc.gpsimd.memset(hpi_sb[:], HALF_PI)
        # dummy Sin pulls the 1.3us ACT_TABLE_LOAD into the prologue
        warm_sb = const_pool.tile([128, 1], F16, tag="warm")
        nc.gpsimd.memset(warm_sb[:], 0.0)
        warm2_sb = const_pool.tile([128, 1], F16, tag="warm2")
        nc.scalar.activation(warm2_sb[:], warm_sb[:], SIN)

        def issue_in_dmas(it):
            bo, nb = ITEMS[it]
            r, lo = bo // 2, bo % 2
            rows = slice(r * 128, (r + 1) * 128)
            xtg = xt_pool.tile([128, nb * KC * 128], F16, tag="xt")
            if it == 0:
                # one DMA per block from the Activation queue: the first
                # matmul waits for a whole block so the PE never starves
                # mid-block
                nc.scalar.dma_start(xtg[:, 0:KC * 128], xt[rows, 0:KC * 128])
                nc.scalar.dma_start(xtg[:, KC * 128:2 * KC * 128],
                                    xt[rows, KC * 128:2 * KC * 128])
            else:
                xsl = slice(lo * KC * 128, (lo + nb) * KC * 128)
                nc.sync.dma_start(xtg[:], xt[rows, xsl])
            return xtg

        def stage_deltas_block(it, xtg, bkl, dp, d16):
            """Delta matmuls + per-block d16 downcast for one 128-t block."""
            sl = slice(bkl * P, (bkl + 1) * P)
            for dc in range(KC):
                nc.tensor.matmul(
                    dp[:, sl],
                    xtg[:, (bkl * KC + dc) * 128:(bkl * KC + dc + 1) * 128],
                    w_sb[:, dc * P:(dc + 1) * P],
                    start=(dc == 0), stop=(dc == KC - 1))
            # PSUM->SBUF fp16 downcast on DVE per block, so the angle
            # matmuls of this item never wait on a whole-pair cast
            nc.vector.tensor_copy(d16[:, sl], dp[:, sl])

        def stage_angles(it, d16):
            """Cumsum + affine matmuls on the PE for item `it`."""
            bo, nb = ITEMS[it]
            wid = nb * P
            ang = ang_pool.tile([128, wid], F32, tag="ang")
            for bkl in range(nb):
                bk = bo + bkl
                sl = slice(bkl * P, (bkl + 1) * P)
                nc.tensor.matmul(ang[:, sl], u_sb[:], d16[:, sl],
                                 start=True, stop=False)
                nc.tensor.matmul(ang[:, sl], afs_sb[:],
                                 afm_sb[:, bk * P:(bk + 1) * P],
                                 start=False, stop=True)
            return ang

        def stage_back(it, ang, xtg):
            """Range reduction + transposes + trig + rotation + out DMA."""
            bo, nb = ITEMS[it]
            wid = nb * P
            # range reduction (turns): rs = y - round(y) in [-0.5, 0.5]
            a_s = a32_pool.tile([128, wid], F32, tag="a_s")
            nc.scalar.activation(a_s[:], ang[:], IDENT,
                                 bias=magic_sb[:], scale=-1.0)
            rs = trig_pool.tile([128, wid], F16, tag="rs")
            nc.vector.scalar_tensor_tensor(rs[:], a_s[:], MAGIC, ang[:],
                                           op0=SUB, op1=ADD)

            # transpose rs to [p, t]: per (block, p-chunk) 128x128 PE
            # transpose; rsT cols = (b, c, t)
            rst = rst_pool.tile([128, nb * 2 * 128], F16, tag="rst")
            for bkl in range(nb):
                for pc in range(2):
                    nc.tensor.transpose(
                        rst[:, (bkl * 2 + pc) * 128:(bkl * 2 + pc + 1) * 128],
                        rs[:, bkl * P + pc * 128:bkl * P + (pc + 1) * 128],
                        id_sb[:])

            sn = trig_pool.tile([128, nb * 2 * 128], F16, tag="sn")
            nc.scalar.activation(sn[:], rst[:], SIN, scale=SCALE_2PI)
            # cos(2pi*y) = sin(pi/2 - 2pi*|rs|), same reduction
            ra = trig_pool.tile([128, nb * 2 * 128], F16, tag="ra")
            nc.scalar.activation(ra[:], rst[:],
                                 mybir.ActivationFunctionType.Abs)
            cs = trig_pool.tile([128, nb * 2 * 128], F16, tag="cs")
            nc.scalar.activation(cs[:], ra[:], SIN,
                                 scale=-SCALE_2PI, bias=hpi_sb[:])

            # rotation in [p, t]: x1^T/x2^T are xt chunks dc 0..1 / 2..3
            xv = xtg[:].rearrange("q (b k t) -> q b k t",
                                  b=nb, k=KC, t=128)
            x1 = xv[:, :, 0:2, :]
            x2 = xv[:, :, 2:4, :]
            snv = sn[:].rearrange("q (b c t) -> q b c t", b=nb, c=2, t=128)
            csv = cs[:].rearrange("q (b c t) -> q b c t", b=nb, c=2, t=128)
            wid2 = nb * 2 * 128
            o = out_pool.tile([128, 2 * wid2], F16, tag="o")
            o1 = o[:, 0:wid2].rearrange("q (c b t) -> q b c t",
                                        c=2, b=nb, t=128)
            o2 = o[:, wid2:2 * wid2].rearrange("q (c b t) -> q b c t",
                                               c=2, b=nb, t=128)
            t1 = rot_pool.tile([128, wid2], F16, tag="t1")
            t1v = t1[:].rearrange("q (b c t) -> q b c t", b=nb, c=2, t=128)
            nc.vector.tensor_mul(t1v, x1, csv)
            t2 = rot_pool.tile([128, wid2], F16, tag="t2")
            t2v = t2[:].rearrange("q (b c t) -> q b c t", b=nb, c=2, t=128)
            nc.vector.tensor_mul(t2v, x2, snv)
            nc.vector.tensor_sub(o1, t1v, t2v)
            t3 = rot_pool.tile([128, wid2], F16, tag="t3")
            t3v = t3[:].rearrange("q (b c t) -> q b c t", b=nb, c=2, t=128)
            nc.vector.tensor_mul(t3v, x2, csv)
            t4 = rot_pool.tile([128, wid2], F16, tag="t4")
            t4v = t4[:].rearrange("q (b c t) -> q b c t", b=nb, c=2, t=128)
            nc.vector.tensor_mul(t4v, x1, snv)
            nc.vector.tensor_add(o2, t3v, t4v)

            off = ITEM_OFF[it]
            nc.gpsimd.dma_start(outT[:, off:off + 4 * nb * 128], o[:])

        # remaining w quarters + angle/transpose constants on SP, behind
        # the first xt DMAs (these are only needed by pair-0's back stage)
        def issue_w_rest():
            for q in range(1, 4):
                nc.sync.dma_start(w_sb[:, q * 4 * P:(q + 1) * 4 * P],
                                  w[:, q * 4 * P:(q + 1) * 4 * P])
            nc.sync.dma_start(u_sb[:], u[:])
            nc.sync.dma_start(id_sb[:], ident[:])
            nc.sync.dma_start(afs_sb[:], afs[:])
            nc.sync.dma_start(afm_sb[:], afm[:])

        # PE emission order per iteration (matching runtime readiness so
        # the in-order PE never waits on a Scalar/DVE round-trip):
        #   angle-MMs(it-1) | deltas(it) block A | transposes(it-1)
        #   | deltas(it) block B
        # with the previous item's trig/rotation interleaved on Scalar/DVE.
        pend = None  # (it, d16, xtg) awaiting angles + back stage
        for it in range(len(ITEMS)):
            xtg = issue_in_dmas(it)
            if it == 0:
                issue_w_rest()
            bo, nb = ITEMS[it]
            dp = dp_pool.tile([128, nb * P], F32, tag="dp")
            d16 = d16_pool.tile([128, nb * P], F16, tag="d16")
            back = None
            if pend is not None:
                back = (pend[0], stage_angles(pend[0], pend[1]), pend[2])
            stage_deltas_block(it, xtg, 0, dp, d16)
            if back is not None:
                stage_back(*back)
            if nb == 2:
                stage_deltas_block(it, xtg, 1, dp, d16)
            pend = (it, d16, xtg)
        back = (pend[0], stage_angles(pend[0], pend[1]), pend[2])
        stage_back(*back)

    nc.compile()
    return nc


_NC_CACHE: dict = {}


def _get_nc():
    if "nc" not in _NC_CACHE:
        _NC_CACHE["nc"] = build_program()
    return _NC_CACHE["nc"]


def prepare_weights(W: np.ndarray, b: np.ndarray):
    inv2pi = 1.0 / (2.0 * np.pi)
    Wt = W.astype(np.float64).T * inv2pi                       # [D, P]
    wh = Wt.astype(np.float16)
    bt = b.astype(np.float64) * inv2pi                         # [P]
    bh = bt.astype(np.float16)
    bl = (bt - bh.astype(np.float64)).astype(np.float16)
    # [D, P] -> [128, KC*P] with d-chunks along the free dim
    w_in = np.ascontiguousarray(
        wh.reshape(KC, 128, P).transpose(1, 0, 2).reshape(128, KC * P))
    # Bases must come from the FULL-precision weights so each 128-step block
    # restarts at the reference-exact angle: the device's fp16-W error then
    # only drifts within one block instead of accumulating across the shard.
    return w_in, bh, bl, Wt, bt


def make_in_maps(x: np.ndarray, W: np.ndarray, b: np.ndarray):
    B, T, _ = x.shape
    w_in, bh, bl, w_eff, b_eff = prepare_weights(W, b)

    u_in = np.triu(np.ones((128, 128), np.float16))
    id_in = np.eye(128, dtype=np.float16)
    afs_in = np.stack([
        np.ones(128, np.float16), np.ones(128, np.float16),
        np.arange(1, 129, dtype=np.float16),
        np.arange(1, 129, dtype=np.float16)])

    # fp64 cumulative angle at every 128-step boundary, per batch (turns)
    nblk = T // BK                                              # 32
    xblk = x.reshape(B, nblk, BK, D).sum(axis=2, dtype=np.float64)
    dblk = xblk @ w_eff + BK * b_eff                            # [B, 32, P]
    bases = np.zeros((B, nblk, P))
    np.cumsum(dblk[:, :-1], axis=1, out=bases[:, 1:])           # exclusive

    in_maps = []
    for c in range(N_CORES):
        bb, hh = c // 2, c % 2
        xs = x[bb, hh * TL:(hh + 1) * TL, :].astype(np.float16)  # [TL, D]
        # xt: [r*128 + dp, (bkl*KC + dc)*128 + tl]
        xt_in = np.ascontiguousarray(
            xs.reshape(NPAIR, 2, BK, KC, 128).transpose(0, 4, 1, 3, 2)
            .reshape(NPAIR * 128, 2 * KC * 128))
        bs = bases[bb, hh * NBK:(hh + 1) * NBK]                 # [NBK, P]
        bs_hi = bs.astype(np.float16)
        bs_lo = (bs - bs_hi.astype(np.float64)).astype(np.float16)
        afm_in = np.stack([
            bs_hi.reshape(NBK * P), bs_lo.reshape(NBK * P),
            np.tile(bh, NBK), np.tile(bl, NBK)])
        in_maps.append({
            "xt": xt_in, "w": w_in, "u": u_in, "ident": id_in,
            "afs": afs_in, "afm": np.ascontiguousarray(afm_in),
        })
    return in_maps


def assemble_output(x: np.ndarray, results) -> np.ndarray:
    B, T, Din = x.shape
    out = np.empty((B, T, Din), np.float32)
    out[:, :, ROT:] = x[:, :, ROT:]
    for c in range(N_CORES):
        bb, hh = c // 2, c % 2
        r = results[c]["outT"]                                # [128, 8192]
        blk = np.empty((TL, ROT), np.float32)
        for it, (bo, nb) in enumerate(ITEMS):
            off = ITEM_OFF[it]
            seg = r[:, off:off + 4 * nb * 128]                # [q, h*c*b*t]
            seg = seg.reshape(128, 2, 2, nb, 128)             # q h c b t
            # rotated[(bo+b)*128 + t, h*256 + c*128 + q]
            seg = seg.transpose(3, 4, 1, 2, 0).reshape(nb * 128, ROT)
            blk[bo * 128:(bo + nb) * 128] = seg
        out[bb, hh * TL:(hh + 1) * TL, :ROT] = blk
    return out


def kernel(x: np.ndarray, W: np.ndarray, b: np.ndarray) -> np.ndarray:
    nc = _get_nc()
    in_maps = make_in_maps(x, W, b)
    res = run_bass_kernel_spmd(nc, in_maps, list(range(N_CORES)))
    return assemble_output(x, res.results)
